# revision 15
# baseline (speedup 1.0000x reference)
"""BRNN-CTC loss kernel on 8 NeuronCores via Bass/Tile.

Strategy: data-parallel over batch B=32 -> 4 sequences/core.
Device computes: input GEMMs, BiLSTM recurrence (fwd+bwd chains),
projection, two CTC heads (softmax via PE column sums), label-prob
gather via one-hot matmul, and the CTC alpha DP in normalized linear
space (packed layout: partition=(chain, S-slice), free=S-within-slice).
Host does: weight repacking, mask/one-hot building, and the final tiny
log-sum reductions (a few hundred floats).
"""
import numpy as np
import ml_dtypes
from contextlib import ExitStack

import concourse.bass as bass
import concourse.bacc as bacc
import concourse.mybir as mybir
import concourse.tile as tile
from concourse import bass_utils

BF16 = mybir.dt.bfloat16
F32 = mybir.dt.float32
AX = mybir.AxisListType
OP = mybir.AluOpType
AF = mybir.ActivationFunctionType

NCORES = 8
T, B, F, H, INNER, V, L = 1024, 32, 128, 128, 512, 64, 200
S = 2 * L + 1            # 401
BPC = B // NCORES        # 4 sequences per core
NCH = 2 * BPC            # 8 CTC chains per core (2 heads x 4 seqs)
KSL = 16                 # S-slices per chain -> 8*16 = 128 partitions
OWN = 26                 # owned S positions per slice (16*26=416 >= 401)
HW = 16                  # left halo width (recomputed, refreshed every RENORM)
WIN = HW + OWN           # alpha window per partition (42)
PJ = WIN - 2             # p-hat cols per partition per step (40)
LPAD = HW                # left zero-pad in gather: col = s + LPAD
SP = LPAD + S + 15       # padded gather width (432)
C_SCALE = 64.0           # anti-drift constant folded into one-hot E
RENORM = 8               # renorm every 8 DP steps
NR = (T - 1) // RENORM + 2


def _np_bf16(x):
    return np.asarray(x, dtype=ml_dtypes.bfloat16)


# ----------------------------------------------------------------- host prep

def _pack_lstm_weights(Wih, Whh, b):
    """Reorder gate blocks to [i, f, o, 2*g] and transpose for lhsT."""
    def blocks(W):
        i, f, g, o = W[0:H], W[H:2*H], W[2*H:3*H], W[3*H:4*H]
        return np.concatenate([i, f, o, 2.0 * g], axis=0)
    Wihb, Whhb, bb = blocks(Wih), blocks(Whh), blocks(b.reshape(4*H, 1))[:, 0]
    wihT = np.concatenate([Wihb[128*g:128*(g+1)].T for g in range(4)], axis=1)
    whhT = np.concatenate([Whhb[128*g:128*(g+1)].T for g in range(4)], axis=1)
    bias = np.stack([bb[128*g:128*(g+1)] for g in range(4)], axis=1)
    return _np_bf16(wihT), _np_bf16(whhT), np.float32(bias)


def _build_ctc_host(tgt, tlen):
    """Per-chain ext labels / masks. tgt:[S-labels row], returns dicts."""
    ext = np.zeros(S, np.int64)
    ext[1::2] = tgt
    skip = np.zeros(S, np.float32)
    sr = np.arange(S)
    skip[(sr % 2 == 1) & (sr >= 2)] = 1.0
    skip[2:][ext[2:] == ext[:-2]] = 0.0
    fin = np.zeros(S, np.float32)
    fin[2 * tlen] = 1.0
    fin[2 * tlen - 1] = 1.0
    return ext, skip, fin


def _prep_core(inputs, core, shared):
    b0 = core * BPC
    x = np.asarray(inputs['inputs'][b0:b0 + BPC], np.float32)
    xT = _np_bf16(x.transpose(2, 0, 1).reshape(H, BPC * T))

    E2 = np.zeros((128, NCH * SP), np.float32)
    maskM = np.zeros((128, PJ), np.float32)
    F_big = np.zeros((128, WIN), np.float32)
    initM = np.zeros((128, PJ), np.float32)
    ratio0 = np.ones((128, 1), np.float32)
    tls = np.zeros(NCH, np.float32)
    for h in range(2):
        tgts = inputs['targets'] if h == 0 else inputs['rles']
        lens = inputs['targets_length'] if h == 0 else inputs['rles_length']
        for bl in range(BPC):
            c = h * BPC + bl
            ext, skip, fin = _build_ctc_host(
                np.asarray(tgts[b0 + bl], np.int64),
                int(lens[b0 + bl]))
            tls[c] = float(lens[b0 + bl])
            E = np.zeros((V, SP), np.float32)
            E[ext, LPAD + np.arange(S)] = C_SCALE
            E2[0:64, c * SP:(c + 1) * SP] = E
            E2[64:128, c * SP:(c + 1) * SP] = E
            for k in range(KSL):
                p = c * KSL + k
                # window position w covers s(w) = 26*k + w - HW, w in [0,WIN)
                for w in range(2, WIN):
                    s = OWN * k + w - HW
                    if 0 <= s < S:
                        maskM[p, w - 2] = skip[s]
                        if s <= 1:
                            initM[p, w - 2] = 1.0
                if k > 0 or True:
                    pass
                s0 = k * OWN
                n = min(OWN, S - s0)
                if n > 0:
                    F_big[p, HW:HW + n] = fin[s0:s0 + n]
            ratio0[c * KSL, 0] = 0.0

    m = dict(shared)
    m.update(xT=xT, E2=_np_bf16(E2), maskM=maskM, F_big=F_big,
             initM=initM, ratio0=ratio0)
    return m, tls


def _prep_shared(inputs):
    wihT_f, whhT_f, bias_f = _pack_lstm_weights(
        np.float32(inputs['W_ih_f']), np.float32(inputs['W_hh_f']),
        np.float32(inputs['b_f']))
    wihT_b, whhT_b, bias_b = _pack_lstm_weights(
        np.float32(inputs['W_ih_b']), np.float32(inputs['W_hh_b']),
        np.float32(inputs['b_b']))

    Wf2 = 2.0 * np.float32(inputs['W_fwd'])           # [INNER, 2H]
    wfT_k1 = np.concatenate(
        [Wf2[128*i:128*(i+1), 0:128].T for i in range(4)], axis=1)
    wfT_k2 = np.concatenate(
        [Wf2[128*i:128*(i+1), 128:256].T for i in range(4)], axis=1)
    bias_fw = np.stack(
        [2.0 * np.float32(inputs['b_fwd'])[128*i:128*(i+1)]
         for i in range(4)], axis=1)

    Wcat = np.concatenate(
        [np.float32(inputs['W_base']), np.float32(inputs['W_rle'])], axis=0)
    whT = np.concatenate(
        [Wcat[:, 128*k:128*(k+1)].T for k in range(4)], axis=1)
    bias_h = np.concatenate(
        [np.float32(inputs['b_base']), np.float32(inputs['b_rle'])]
    ).reshape(128, 1)

    ident = _np_bf16(np.eye(128, dtype=np.float32))
    sel = np.zeros((128, 2), np.float32)
    sel[0:64, 0] = 1.0
    sel[64:128, 1] = 1.0
    P1 = np.zeros((128, 128), np.float32)     # out[m] = in[m-1]
    P1[np.arange(127), np.arange(1, 128)] = 1.0

    return dict(
        wihT_f=wihT_f, whhT_f=whhT_f, bias_f=np.float32(bias_f),
        wihT_b=wihT_b, whhT_b=whhT_b, bias_b=np.float32(bias_b),
        wfT_k1=_np_bf16(wfT_k1), wfT_k2=_np_bf16(wfT_k2),
        bias_fw=np.float32(bias_fw),
        whT=_np_bf16(whT), bias_h=np.float32(bias_h),
        ident=ident, sel=_np_bf16(sel), P1=np.float32(P1))


# ------------------------------------------------------------- device kernel

IN_SPECS = [
    ('xT', (H, BPC * T), BF16),
    ('wihT_f', (128, 512), BF16), ('whhT_f', (128, 512), BF16),
    ('bias_f', (128, 4), F32),
    ('wihT_b', (128, 512), BF16), ('whhT_b', (128, 512), BF16),
    ('bias_b', (128, 4), F32),
    ('wfT_k1', (128, 512), BF16), ('wfT_k2', (128, 512), BF16),
    ('bias_fw', (128, 4), F32),
    ('whT', (128, 512), BF16), ('bias_h', (128, 1), F32),
    ('ident', (128, 128), BF16), ('sel', (128, 2), BF16),
    ('P1', (128, 128), F32),
    ('E2', (128, NCH * SP), BF16),
    ('maskM', (128, PJ), F32), ('F_big', (128, WIN), F32),
    ('initM', (128, PJ), F32), ('ratio0', (128, 1), F32),
]
OUT_SPECS = [
    ('d_out', (128, 1), F32),
    ('ms_out', (128, NR), F32),
]


def build_program():
    nc = bacc.Bacc(
        "TRN2", target_bir_lowering=False, debug=False,
        enable_asserts=False, num_devices=NCORES)
    ins = {n: nc.dram_tensor(n, list(s), d, kind="ExternalInput").ap()
           for n, s, d in IN_SPECS}
    outs = {n: nc.dram_tensor(n, list(s), d, kind="ExternalOutput").ap()
            for n, s, d in OUT_SPECS}
    with tile.TileContext(nc) as tc:
        _emit(tc, outs, ins)
    nc.finalize()
    return nc


def _emit(tc, outs, ins):
    nc = tc.nc
    NT = BPC * T               # total (b, t) columns
    NCK = NT // 512            # 512-col chunks
    with ExitStack() as ctx:
        const = ctx.enter_context(tc.tile_pool(name="const", bufs=1))

        def load(name, dtype=None):
            return const.tile_from(ins[name], name=name + "_sb", dtype=dtype)

        xT = load('xT')
        wihT = {0: load('wihT_f'), 1: load('wihT_b')}
        whhT = {0: load('whhT_f'), 1: load('whhT_b')}
        bias = {0: load('bias_f'), 1: load('bias_b')}
        wfT = {0: load('wfT_k1'), 1: load('wfT_k2')}
        bias_fw = load('bias_fw')
        whT = load('whT')
        bias_h = load('bias_h')
        ident = load('ident')
        sel = load('sel')
        P1 = load('P1')
        E2 = load('E2')
        maskM = load('maskM')
        F_big = load('F_big')
        initM = load('initM')

        # big SBUF state
        xw = {d: const.tile([128, 4 * NT], BF16, name=f"xw{d}")
              for d in range(2)}
        hst = {d: const.tile([128, NT], BF16, name=f"h{d}") for d in range(2)}
        fo = [const.tile([128, NT], BF16, name=f"fo{i}") for i in range(4)]

        # ---- phase 1: xW = Wih @ x (+bias), bf16, col = g*NT + b*T + t
        with tc.tile_pool(name="ps1", bufs=4, space="PSUM") as ps1:
            for d in range(2):
                for g in range(4):
                    for ck in range(NCK):
                        ps = ps1.tile([128, 512], F32, name="p1")
                        nc.tensor.matmul(
                            ps, lhsT=wihT[d][:, 128*g:128*(g+1)],
                            rhs=xT[:, 512*ck:512*(ck+1)],
                            start=True, stop=True)
                        nc.vector.tensor_scalar(
                            xw[d][:, g*NT + 512*ck: g*NT + 512*(ck+1)],
                            ps, bias[d][:, g:g+1], None, OP.add)

        # ---- phase 2: the BiLSTM recurrence (fwd chain d=0, bwd d=1)
        xw_r = {d: xw[d].rearrange("p (g b t) -> p g b t", g=4, b=BPC)
                for d in range(2)}
        h_r = {d: hst[d].rearrange("p (b t) -> p b t", b=BPC)
               for d in range(2)}
        with tc.tile_pool(name="ps2", bufs=2, space="PSUM") as ps2, \
             tc.tile_pool(name="lwork", bufs=3) as lw:
            cst = {d: const.tile([128, BPC], F32, name=f"c{d}")
                   for d in range(2)}
            for d in range(2):
                nc.vector.memset(cst[d][:], 0.0)
            for i in range(T):
                for d in range(2):
                    t = i if d == 0 else T - 1 - i
                    tp = t - 1 if d == 0 else t + 1
                    ps = ps2.tile([128, 16], F32, name=f"psg{d}", tag=f"psg{d}")
                    nc.tensor.matmul(
                        ps, lhsT=ident[:], rhs=xw_r[d][:, :, :, t],
                        start=True, stop=(i == 0))
                    if i > 0:
                        for g in range(4):
                            nc.tensor.matmul(
                                ps[:, 4*g:4*(g+1)],
                                lhsT=whhT[d][:, 128*g:128*(g+1)],
                                rhs=h_r[d][:, :, tp],
                                start=False, stop=(g == 3))
                    ghat = lw.tile([128, 16], F32, name="ghat", tag="ghat")
                    nc.scalar.activation(ghat[:], ps, AF.Sigmoid)
                    gt = lw.tile([128, 4], F32, name="gt", tag="gt")
                    nc.vector.tensor_scalar(
                        gt[:], ghat[:, 12:16], 2.0, -1.0, OP.mult, OP.add)
                    t1 = lw.tile([128, 4], F32, name="t1", tag="t1")
                    nc.vector.tensor_tensor(t1[:], ghat[:, 0:4], gt[:], OP.mult)
                    u = lw.tile([128, 4], F32, name="u", tag="u")
                    nc.vector.tensor_tensor(u[:], ghat[:, 4:8], cst[d][:], OP.mult)
                    nc.vector.tensor_tensor(cst[d][:], u[:], t1[:], OP.add)
                    sh = lw.tile([128, 4], F32, name="sh", tag="sh")
                    nc.scalar.activation(sh[:], cst[d][:], AF.Sigmoid, scale=2.0)
                    st = lw.tile([128, 4], F32, name="st", tag="st")
                    nc.vector.tensor_scalar(
                        st[:], sh[:], 2.0, -1.0, OP.mult, OP.add)
                    nc.vector.tensor_tensor(
                        h_r[d][:, :, t], ghat[:, 8:12], st[:], OP.mult)

        # ---- phase 3: fo = tanh(Wf enc + b) via 2*sigmoid(2x)-1
        with tc.tile_pool(name="ps3", bufs=4, space="PSUM") as ps3, \
             tc.tile_pool(name="pwork", bufs=3) as pw:
            for ck in range(NCK):
                for isl in range(4):
                    ps = ps3.tile([128, 512], F32, name="p3")
                    nc.tensor.matmul(
                        ps, lhsT=wfT[0][:, 128*isl:128*(isl+1)],
                        rhs=hst[0][:, 512*ck:512*(ck+1)],
                        start=True, stop=False)
                    nc.tensor.matmul(
                        ps, lhsT=wfT[1][:, 128*isl:128*(isl+1)],
                        rhs=hst[1][:, 512*ck:512*(ck+1)],
                        start=False, stop=True)
                    sp_ = pw.tile([128, 512], F32, name="sp", tag="sp")
                    nc.scalar.activation(
                        sp_[:], ps, AF.Sigmoid, bias=bias_fw[:, isl:isl+1])
                    nc.vector.tensor_scalar(
                        fo[isl][:, 512*ck:512*(ck+1)],
                        sp_[:], 2.0, -1.0, OP.mult, OP.add)

        # table-set switch fence (sigmoid set -> exp set)
        tc.strict_bb_all_engine_barrier()

        # ---- phase 4: heads, softmax pieces, gather, relayout to DRAM
        with tc.tile_pool(name="dram", bufs=1, space="DRAM") as dp:
            phD = dp.tile([128, T * PJ], F32, name="phD")
            phD_r = phD.rearrange("r (t j) -> r t j", j=PJ)
            with tc.tile_pool(name="ps4", bufs=2, space="PSUM") as ps4, \
                 tc.tile_pool(name="ps5", bufs=2, space="PSUM") as ps5, \
                 tc.tile_pool(name="ps6", bufs=2, space="PSUM") as ps6, \
                 tc.tile_pool(name="hwork", bufs=2) as hw:
                for ck in range(NCK):
                    psl = ps4.tile([128, 512], F32, name="p4")
                    for k in range(4):
                        nc.tensor.matmul(
                            psl, lhsT=whT[:, 128*k:128*(k+1)],
                            rhs=fo[k][:, 512*ck:512*(ck+1)],
                            start=(k == 0), stop=(k == 3))
                    ut = hw.tile([128, 512], BF16, name="ut", tag="ut")
                    nc.scalar.activation(ut[:], psl, AF.Exp, bias=bias_h[:])
                    psz = ps5.tile([128, 8], F32, name="p5", tag="p5")
                    for tl in range(4):
                        nc.tensor.matmul(
                            psz[:, 2*tl:2*(tl+1)],
                            lhsT=ut[:, 128*tl:128*(tl+1)], rhs=sel[:],
                            start=True, stop=True)
                    zi = hw.tile([128, 8], F32, name="zi", tag="zi")
                    nc.vector.reciprocal(zi[:], psz)
                    for tl in range(4):
                        g0 = ck * 512 + tl * 128
                        bl, t0 = g0 // T, g0 % T
                        for h in range(2):
                            c = h * BPC + bl
                            psg = ps6.tile([128, SP], F32, name="p6",
                                           tag="p6")
                            nc.tensor.matmul(
                                psg,
                                lhsT=ut[64*h:64*(h+1), 128*tl:128*(tl+1)],
                                rhs=E2[64*h:64*(h+1), c*SP:(c+1)*SP],
                                start=True, stop=True)
                            stg = hw.tile([128, SP], F32, name="stg",
                                          tag="stg")
                            nc.vector.tensor_scalar(
                                stg[:], psg, zi[:, 2*tl+h:2*tl+h+1],
                                None, OP.mult)
                            # window w in [2, WIN) <-> stage col OWN*k + w
                            dst = phD_r[c*KSL:(c+1)*KSL, t0:t0+128, :]\
                                .rearrange("k t j -> t k j")
                            src_o = stg[:, HW:HW+KSL*OWN].rearrange(
                                "p (k j) -> p k j", j=OWN)
                            nc.sync.dma_start(
                                dst[:, :, HW-2:PJ], src_o)
                            src_h = stg[:, 2:2+KSL*OWN].rearrange(
                                "p (k j) -> p k j", j=OWN)[:, :, 0:HW-2]
                            nc.sync.dma_start(
                                dst[:, :, 0:HW-2], src_h)

            # ---- phase 5: CTC alpha DP, packed layout with recomputed halo
            CH = 64                       # DP steps per streamed chunk
            with tc.tile_pool(name="pch", bufs=2) as pcp, \
                 tc.tile_pool(name="psD", bufs=2, space="PSUM") as psD, \
                 tc.tile_pool(name="dwork", bufs=1) as dw:
                alpha = dw.tile([128, WIN], F32, name="alpha")
                wt = dw.tile([128, WIN], F32, name="wt")
                vt = dw.tile([128, PJ], F32, name="vt")
                ratio = const.tile_from(ins['ratio0'], name="ratio_sb")
                msb = dw.tile([128, NR], F32, name="msb")
                minv = dw.tile([128, 1], F32, name="minv")
                mprev = dw.tile([128, 1], F32, name="mprev")
                d_sb = dw.tile([128, 1], F32, name="d_sb")
                nc.vector.memset(alpha[:], 0.0)
                nc.vector.memset(wt[:], 0.0)
                nc.vector.memset(msb[:], 1.0)

                pch = None
                for t in range(T):
                    if t % CH == 0:
                        pch = pcp.tile([128, PJ * CH], F32, name="pch",
                                       tag="pch")
                        nc.sync.dma_start(
                            pch[:], phD[:, t*PJ:(t+CH)*PJ])
                    pt = pch[:, (t % CH)*PJ:(t % CH + 1)*PJ]
                    if t == 0:
                        nc.vector.tensor_tensor(
                            alpha[:, 2:WIN], pt, initM[:], OP.mult)
                    else:
                        nc.vector.tensor_tensor(
                            vt[:], maskM[:], alpha[:, 0:PJ], OP.mult)
                        nc.vector.tensor_tensor(
                            wt[:, 2:WIN], alpha[:, 2:WIN],
                            alpha[:, 1:WIN-1], OP.add)
                        nc.vector.tensor_tensor(
                            wt[:, 2:WIN], wt[:, 2:WIN], vt[:], OP.add)
                        nc.vector.tensor_tensor(
                            alpha[:, 2:WIN], wt[:, 2:WIN], pt, OP.mult)
                    if t % RENORM == 0 and t > 0 and t < T - 1:
                        r = t // RENORM
                        # renorm down by per-partition max (guarded >= 1)
                        nc.vector.tensor_reduce(
                            msb[:, r:r+1], alpha[:, HW:WIN], AX.X, OP.max)
                        nc.vector.tensor_scalar(
                            msb[:, r:r+1], msb[:, r:r+1], 1.0, None, OP.max)
                        nc.vector.reciprocal(minv[:], msb[:, r:r+1])
                        nc.vector.tensor_scalar(
                            alpha[:], alpha[:], minv[:], None, OP.mult)
                        # ratio[p] *= m'[p-1] * minv[p]  (shift via PE)
                        psm = psD.tile([128, 1], F32, name="psm", tag="psm")
                        nc.tensor.matmul(
                            psm, lhsT=P1[:], rhs=msb[:, r:r+1],
                            start=True, stop=True)
                        nc.vector.tensor_tensor(
                            mprev[:], ratio[:], psm, OP.mult)
                        nc.vector.tensor_tensor(
                            ratio[:], mprev[:], minv[:], OP.mult)
                        # refresh halo from left neighbour (shift via PE)
                        psh = psD.tile([128, HW], F32, name="psh", tag="psh")
                        nc.tensor.matmul(
                            psh, lhsT=P1[:], rhs=alpha[:, OWN:WIN],
                            start=True, stop=True)
                        nc.vector.tensor_scalar(
                            alpha[:, 0:HW], psh, ratio[:], None, OP.mult)

                nc.vector.scalar_tensor_tensor(
                    wt[:], alpha[:], 1.0, F_big[:], OP.mult, OP.mult,
                    accum_out=d_sb[:])
                nc.sync.dma_start(outs['d_out'], d_sb[:])
                nc.sync.dma_start(outs['ms_out'], msb[:])


# ----------------------------------------------------------------- interface

_CACHE = {}


def _get_program():
    if 'nc' not in _CACHE:
        _CACHE['nc'] = build_program()
    return _CACHE['nc']


def kernel(**inputs):
    assert np.all(np.asarray(inputs['inputs_length']) == T), \
        "kernel assumes full-length inputs"
    nc = _get_program()
    shared = _prep_shared(inputs)
    in_maps, tls_all = [], []
    for core in range(NCORES):
        m, tls = _prep_core(inputs, core, shared)
        in_maps.append(m)
        tls_all.append(tls)

    res = bass_utils.run_bass_kernel_spmd(
        nc, in_maps, core_ids=list(range(NCORES)))

    base_losses, rle_losses = [], []
    for core in range(NCORES):
        d = np.float64(res.results[core]['d_out'][:, 0])
        ms = np.float64(res.results[core]['ms_out'])
        lam = np.sum(np.log(ms), axis=1)          # [128] log Lambda_p
        tls = tls_all[core]
        for h in range(2):
            for bl in range(BPC):
                c = h * BPC + bl
                dk = d[c*KSL:(c+1)*KSL]
                lk = lam[c*KSL:(c+1)*KSL]
                good = dk > 0
                if not np.any(good):
                    ll = -np.inf
                else:
                    terms = np.log(dk[good]) + lk[good]
                    mx = np.max(terms)
                    ll = mx + np.log(np.sum(np.exp(terms - mx)))
                ll -= T * np.log(C_SCALE)
                loss = -ll / tls[c]
                (base_losses if h == 0 else rle_losses).append(loss)
    return np.asarray(
        [np.mean(base_losses), np.mean(rle_losses)], np.float32)


# revision 40
# speedup vs baseline: 8.3807x; 8.3807x over previous
"""BRNN-CTC loss kernel on 8 NeuronCores via Bass/Tile.

Strategy: data-parallel over batch B=32 -> 4 sequences/core.
Device computes: input GEMMs, BiLSTM recurrence (fwd+bwd chains),
projection, two CTC heads (softmax via PE column sums), label-prob
gather via one-hot matmul, and the CTC alpha DP in normalized linear
space (packed layout: partition=(chain, S-slice), free=S-within-slice).
Host does: weight repacking, mask/one-hot building, and the final tiny
log-sum reductions (a few hundred floats).
"""
import numpy as np
import ml_dtypes
from contextlib import ExitStack

import concourse.bass as bass
import concourse.bacc as bacc
import concourse.mybir as mybir
import concourse.tile as tile
from concourse.tile_rust import add_dep_helper
from concourse import bass_utils

BF16 = mybir.dt.bfloat16
F32 = mybir.dt.float32
AX = mybir.AxisListType
OP = mybir.AluOpType
AF = mybir.ActivationFunctionType

NCORES = 8
T, B, F, H, INNER, V, L = 1024, 32, 128, 128, 512, 64, 200
S = 2 * L + 1            # 401
BPC = B // NCORES        # 4 sequences per core
NCH = 2 * BPC            # 8 CTC chains per core (2 heads x 4 seqs)
KSL = 16                 # S-slices per chain -> 8*16 = 128 partitions
OWN = 26                 # owned S positions per slice (16*26=416 >= 401)
HW = 16                  # left halo width (recomputed, refreshed every RENORM)
WIN = HW + OWN           # alpha window per partition (42)
PJ = WIN - 2             # p-hat cols per partition per step (40)
LPAD = HW                # left zero-pad in gather: col = s + LPAD
SP = LPAD + S + 15       # padded gather width (432)
C_SCALE = 64.0           # anti-drift constant folded into one-hot E
RENORM = 8               # renorm every 8 DP steps
NR = (T - 1) // RENORM + 2


def _np_bf16(x):
    return np.asarray(x, dtype=ml_dtypes.bfloat16)


# ----------------------------------------------------------------- host prep

def _pack_lstm_weights(Wih, Whh, b):
    """Reorder gate blocks to [i, f, o, 2*g] and transpose for lhsT."""
    def blocks(W):
        i, f, g, o = W[0:H], W[H:2*H], W[2*H:3*H], W[3*H:4*H]
        return np.concatenate([i, f, o, 2.0 * g], axis=0)
    Wihb, Whhb, bb = blocks(Wih), blocks(Whh), blocks(b.reshape(4*H, 1))[:, 0]
    wihT = np.concatenate([Wihb[128*g:128*(g+1)].T for g in range(4)], axis=1)
    whhT = np.concatenate([Whhb[128*g:128*(g+1)].T for g in range(4)], axis=1)
    bias = np.stack([bb[128*g:128*(g+1)] for g in range(4)], axis=1)
    return _np_bf16(wihT), _np_bf16(whhT), np.float32(bias)


def _build_ctc_host(tgt, tlen):
    """Per-chain ext labels / masks. tgt:[S-labels row], returns dicts."""
    ext = np.zeros(S, np.int64)
    ext[1::2] = tgt
    skip = np.zeros(S, np.float32)
    sr = np.arange(S)
    skip[(sr % 2 == 1) & (sr >= 2)] = 1.0
    skip[2:][ext[2:] == ext[:-2]] = 0.0
    fin = np.zeros(S, np.float32)
    fin[2 * tlen] = 1.0
    fin[2 * tlen - 1] = 1.0
    return ext, skip, fin


def _prep_core(inputs, core, shared):
    b0 = core * BPC
    x = np.asarray(inputs['inputs'][b0:b0 + BPC], np.float32)
    xT = _np_bf16(x.transpose(2, 0, 1).reshape(H, BPC * T))

    E2 = np.zeros((128, NCH * SP), np.float32)
    maskM = np.zeros((128, PJ), np.float32)
    F_big = np.zeros((128, WIN), np.float32)
    initM = np.zeros((128, PJ), np.float32)
    ratio0 = np.ones((128, 1), np.float32)
    tls = np.zeros(NCH, np.float32)
    for h in range(2):
        tgts = inputs['targets'] if h == 0 else inputs['rles']
        lens = inputs['targets_length'] if h == 0 else inputs['rles_length']
        for bl in range(BPC):
            c = h * BPC + bl
            ext, skip, fin = _build_ctc_host(
                np.asarray(tgts[b0 + bl], np.int64),
                int(lens[b0 + bl]))
            tls[c] = float(lens[b0 + bl])
            E = np.zeros((V, SP), np.float32)
            E[ext, LPAD + np.arange(S)] = C_SCALE
            E2[0:64, c * SP:(c + 1) * SP] = E
            E2[64:128, c * SP:(c + 1) * SP] = E
            for k in range(KSL):
                p = c * KSL + k
                # window position w covers s(w) = 26*k + w - HW, w in [0,WIN)
                for w in range(2, WIN):
                    s = OWN * k + w - HW
                    if 0 <= s < S:
                        maskM[p, w - 2] = skip[s]
                        if s <= 1:
                            initM[p, w - 2] = 1.0
                if k > 0 or True:
                    pass
                s0 = k * OWN
                n = min(OWN, S - s0)
                if n > 0:
                    F_big[p, HW:HW + n] = fin[s0:s0 + n]
            ratio0[c * KSL, 0] = 0.0

    m = dict(shared)
    m.update(xT=xT, E2=_np_bf16(E2), maskM=maskM, F_big=F_big,
             initM=initM, ratio0=ratio0)
    return m, tls


def _prep_shared(inputs):
    wihT_f, whhT_f, bias_f = _pack_lstm_weights(
        np.float32(inputs['W_ih_f']), np.float32(inputs['W_hh_f']),
        np.float32(inputs['b_f']))
    wihT_b, whhT_b, bias_b = _pack_lstm_weights(
        np.float32(inputs['W_ih_b']), np.float32(inputs['W_hh_b']),
        np.float32(inputs['b_b']))

    Wf2 = 2.0 * np.float32(inputs['W_fwd'])           # [INNER, 2H]
    wfT_k1 = np.concatenate(
        [Wf2[128*i:128*(i+1), 0:128].T for i in range(4)], axis=1)
    wfT_k2 = np.concatenate(
        [Wf2[128*i:128*(i+1), 128:256].T for i in range(4)], axis=1)
    bias_fw = np.stack(
        [2.0 * np.float32(inputs['b_fwd'])[128*i:128*(i+1)]
         for i in range(4)], axis=1)

    Wcat = np.concatenate(
        [np.float32(inputs['W_base']), np.float32(inputs['W_rle'])], axis=0)
    whT = np.concatenate(
        [Wcat[:, 128*k:128*(k+1)].T for k in range(4)], axis=1)
    bias_h = np.concatenate(
        [np.float32(inputs['b_base']), np.float32(inputs['b_rle'])]
    ).reshape(128, 1)

    ident = _np_bf16(np.eye(128, dtype=np.float32))
    sel = np.zeros((128, 2), np.float32)
    sel[0:64, 0] = 1.0
    sel[64:128, 1] = 1.0
    P1 = np.zeros((128, 128), np.float32)     # out[m] = in[m-1]
    P1[np.arange(127), np.arange(1, 128)] = 1.0

    return dict(
        wihT_f=wihT_f, whhT_f=whhT_f, bias_f=np.float32(bias_f),
        wihT_b=wihT_b, whhT_b=whhT_b, bias_b=np.float32(bias_b),
        wfT_k1=_np_bf16(wfT_k1), wfT_k2=_np_bf16(wfT_k2),
        bias_fw=np.float32(bias_fw),
        whT=_np_bf16(whT), bias_h=np.float32(bias_h),
        ident=ident, sel=_np_bf16(sel), P1=np.float32(P1))


# ------------------------------------------------------------- device kernel

IN_SPECS = [
    ('xT', (H, BPC * T), BF16),
    ('wihT_f', (128, 512), BF16), ('whhT_f', (128, 512), BF16),
    ('bias_f', (128, 4), F32),
    ('wihT_b', (128, 512), BF16), ('whhT_b', (128, 512), BF16),
    ('bias_b', (128, 4), F32),
    ('wfT_k1', (128, 512), BF16), ('wfT_k2', (128, 512), BF16),
    ('bias_fw', (128, 4), F32),
    ('whT', (128, 512), BF16), ('bias_h', (128, 1), F32),
    ('ident', (128, 128), BF16), ('sel', (128, 2), BF16),
    ('P1', (128, 128), F32),
    ('E2', (128, NCH * SP), BF16),
    ('maskM', (128, PJ), F32), ('F_big', (128, WIN), F32),
    ('initM', (128, PJ), F32), ('ratio0', (128, 1), F32),
]
OUT_SPECS = [
    ('d_out', (128, 1), F32),
    ('ms_out', (128, NR), F32),
]


def build_program(stop_after=99):
    nc = bacc.Bacc(
        "TRN2", target_bir_lowering=False, debug=False,
        enable_asserts=False, num_devices=NCORES)
    ins = {n: nc.dram_tensor(n, list(s), d, kind="ExternalInput").ap()
           for n, s, d in IN_SPECS}
    outs = {n: nc.dram_tensor(n, list(s), d, kind="ExternalOutput").ap()
            for n, s, d in OUT_SPECS}
    with tile.TileContext(nc) as tc:
        _emit(tc, outs, ins, stop_after)
    nc.finalize()
    return nc


def _emit(tc, outs, ins, stop_after=99):
    nc = tc.nc
    NT = BPC * T               # total (b, t) columns
    NCK = NT // 512            # 512-col chunks
    with ExitStack() as ctx:
        const = ctx.enter_context(tc.tile_pool(name="const", bufs=1))

        def load(name, dtype=None):
            return const.tile_from(ins[name], name=name + "_sb", dtype=dtype)

        xT = load('xT')
        wihT = {0: load('wihT_f'), 1: load('wihT_b')}
        whhT = {0: load('whhT_f'), 1: load('whhT_b')}
        bias = {0: load('bias_f'), 1: load('bias_b')}
        wfT = {0: load('wfT_k1'), 1: load('wfT_k2')}
        bias_fw = load('bias_fw')
        whT = load('whT')
        bias_h = load('bias_h')
        ident = load('ident')
        sel = load('sel')
        P1 = load('P1')
        E2 = load('E2')
        maskM = load('maskM')
        F_big = load('F_big')
        initM = load('initM')

        # big SBUF state
        # xw col = g*(2*NT) + d*NT + b*T + t   (g in [i,f,o,2g])
        xw = const.tile([128, 8 * NT], BF16, name="xw")
        # h col = d*NT + b*T + t
        hst = const.tile([128, 2 * NT], BF16, name="hst")
        fo = [const.tile([128, NT], BF16, name=f"fo{i}") for i in range(4)]

        # ---- phase 1: xW = Wih @ x (+bias), bf16
        with tc.tile_pool(name="ps1", bufs=4, space="PSUM") as ps1:
            for d in range(2):
                for g in range(4):
                    for ck in range(NCK):
                        ps = ps1.tile([128, 512], F32, name="p1")
                        nc.tensor.matmul(
                            ps, lhsT=wihT[d][:, 128*g:128*(g+1)],
                            rhs=xT[:, 512*ck:512*(ck+1)],
                            start=True, stop=True)
                        c0 = (2*g + d) * NT + 512*ck
                        nc.vector.tensor_scalar(
                            xw[:, c0:c0+512],
                            ps, bias[d][:, g:g+1], None, OP.add)

        def _dummy_out(*src_aps):
            with tc.tile_pool(name="dumm", bufs=1) as dm:
                for i, ap in enumerate(src_aps):
                    tdum = dm.tile([128, 1], F32, name=f"tdum{i}")
                    nc.vector.tensor_copy(tdum[:], ap)
                    dst = outs['d_out'] if i == 0 else outs['ms_out'][:, i-1:i]
                    nc.sync.dma_start(dst, tdum[:])

        if stop_after <= 1:
            _dummy_out(xw[:, 0:1], xw[:, 8*NT-1:8*NT])
            return

        # ---- phase 2: BiLSTM recurrence. Two independent chains (fwd d=0,
        # bwd d=1) emitted on a hand-placed modulo schedule (half-period
        # offset) so their engine bursts interleave instead of colliding.
        # xw col = g*(2*NT) + d*NT + b*T + t; psum col (per dir) = g*4 + b.
        xw_r = xw.rearrange("p (g d b t) -> p g d b t", g=4, d=2, b=BPC)
        h_r = hst.rearrange("p (d b t) -> p d b t", d=2, b=BPC)
        with tc.tile_pool(name="ps2", bufs=3, space="PSUM") as ps2, \
             tc.tile_pool(name="lwork", bufs=4) as lw:
            cst = {d: const.tile([128, BPC], F32, name=f"c{d}")
                   for d in range(2)}
            for d in range(2):
                nc.vector.memset(cst[d][:], 0.0)

            psums, ghats, shs = {}, {}, {}
            _last = {}

            def _ord(key, binst):
                # force per-engine issue order to match emission order
                prev = _last.get(key)
                if prev is not None:
                    add_dep_helper(binst.ins, prev.ins, sync=False,
                                   reason="modulo-order")
                _last[key] = binst
                return binst

            def part_mm(d, i):
                t = i if d == 0 else T - 1 - i
                tp = t - 1 if d == 0 else t + 1
                ps = ps2.tile([128, 16], F32, name=f"psg{d}", tag=f"psg{d}")
                _ord("PE", nc.tensor.matmul(
                    ps, lhsT=ident[:], rhs=xw_r[:, :, d, :, t],
                    start=True, stop=(i == 0)))
                if i > 0:
                    for g in range(4):
                        _ord("PE", nc.tensor.matmul(
                            ps[:, 4*g:4*(g+1)],
                            lhsT=whhT[d][:, 128*g:128*(g+1)],
                            rhs=h_r[:, d, :, tp],
                            start=False, stop=(g == 3)))
                psums[d] = ps

            def part_sig(d, i):
                ghat = lw.tile([128, 16], F32, name=f"ghat{d}",
                               tag=f"ghat{d}")
                _ord("ACT", nc.scalar.activation(ghat[:], psums[d],
                                                 AF.Sigmoid))
                ghats[d] = ghat

            def part_cpath(d, i):
                ghat = ghats[d]
                u = lw.tile([128, 4], F32, name=f"u{d}", tag=f"u{d}")
                _ord("DVE", nc.vector.tensor_tensor(
                    u[:], ghat[:, 4:8], cst[d][:], OP.mult))
                gt = lw.tile([128, 4], F32, name=f"gt{d}", tag=f"gt{d}")
                _ord("DVE", nc.vector.tensor_scalar(
                    gt[:], ghat[:, 12:16], 2.0, -1.0, OP.mult, OP.add))
                t1 = lw.tile([128, 4], F32, name=f"t1{d}", tag=f"t1{d}")
                _ord("DVE", nc.vector.tensor_tensor(
                    t1[:], ghat[:, 0:4], gt[:], OP.mult))
                _ord("DVE", nc.vector.tensor_tensor(
                    cst[d][:], u[:], t1[:], OP.add))

            def part_sigc(d, i):
                sh = lw.tile([128, 4], F32, name=f"sh{d}", tag=f"sh{d}")
                _ord("ACT", nc.scalar.activation(
                    sh[:], cst[d][:], AF.Tanh))
                shs[d] = sh

            def part_hpath(d, i):
                t = i if d == 0 else T - 1 - i
                _ord("DVE", nc.vector.tensor_tensor(
                    h_r[:, 0 if d == 0 else 1, :, t],
                    ghats[d][:, 8:12], shs[d][:], OP.mult))

            # modulo schedule, one period per step index i (phi order):
            # MM_f(0) hpath_b(135) MM_b(350) sig_f(520) cpath_f(745)
            # sig_b(870) cpath_b(1095) sigc_f(1230) hpath_f(1485) sigc_b(1580)
            part_mm(0, 0)
            part_mm(1, 0)
            part_sig(0, 0)
            part_cpath(0, 0)
            part_sig(1, 0)
            part_cpath(1, 0)
            part_sigc(0, 0)
            part_hpath(0, 0)
            part_sigc(1, 0)
            for i in range(1, T):
                part_mm(0, i)
                part_hpath(1, i - 1)
                part_mm(1, i)
                part_sig(0, i)
                part_cpath(0, i)
                part_sig(1, i)
                part_cpath(1, i)
                part_sigc(0, i)
                part_hpath(0, i)
                part_sigc(1, i)
            part_hpath(1, T - 1)

        if stop_after <= 2:
            _dummy_out(hst[:, NT-1:NT], hst[:, NT:NT+1])
            return

        # ---- phase 3: fo = tanh(Wf enc + b) via 2*sigmoid(2x)-1
        with tc.tile_pool(name="ps3", bufs=4, space="PSUM") as ps3, \
             tc.tile_pool(name="pwork", bufs=3) as pw:
            for ck in range(NCK):
                for isl in range(4):
                    ps = ps3.tile([128, 512], F32, name="p3")
                    nc.tensor.matmul(
                        ps, lhsT=wfT[0][:, 128*isl:128*(isl+1)],
                        rhs=hst[:, 512*ck:512*(ck+1)],
                        start=True, stop=False)
                    nc.tensor.matmul(
                        ps, lhsT=wfT[1][:, 128*isl:128*(isl+1)],
                        rhs=hst[:, NT + 512*ck:NT + 512*(ck+1)],
                        start=False, stop=True)
                    sp_ = pw.tile([128, 512], F32, name="sp", tag="sp")
                    nc.scalar.activation(
                        sp_[:], ps, AF.Sigmoid, bias=bias_fw[:, isl:isl+1])
                    nc.vector.tensor_scalar(
                        fo[isl][:, 512*ck:512*(ck+1)],
                        sp_[:], 2.0, -1.0, OP.mult, OP.add)

        if stop_after <= 3:
            _dummy_out(fo[0][:, 0:1], fo[3][:, NT-1:NT])
            return

        # table-set switch fence (sigmoid set -> exp set)
        tc.strict_bb_all_engine_barrier()

        # ---- phase 4: heads, softmax pieces, gather, relayout to DRAM
        with tc.tile_pool(name="dram", bufs=1, space="DRAM") as dp:
            phD = dp.tile([128, T * PJ], F32, name="phD")
            phD_r = phD.rearrange("r (t j) -> r t j", j=PJ)
            with tc.tile_pool(name="ps4", bufs=2, space="PSUM") as ps4, \
                 tc.tile_pool(name="ps5", bufs=2, space="PSUM") as ps5, \
                 tc.tile_pool(name="ps6", bufs=2, space="PSUM") as ps6, \
                 tc.tile_pool(name="hwork", bufs=2) as hw:
                for ck in range(NCK):
                    psl = ps4.tile([128, 512], F32, name="p4")
                    for k in range(4):
                        nc.tensor.matmul(
                            psl, lhsT=whT[:, 128*k:128*(k+1)],
                            rhs=fo[k][:, 512*ck:512*(ck+1)],
                            start=(k == 0), stop=(k == 3))
                    ut = hw.tile([128, 512], BF16, name="ut", tag="ut")
                    nc.scalar.activation(ut[:], psl, AF.Exp, bias=bias_h[:])
                    psz = ps5.tile([128, 8], F32, name="p5", tag="p5")
                    for tl in range(4):
                        nc.tensor.matmul(
                            psz[:, 2*tl:2*(tl+1)],
                            lhsT=ut[:, 128*tl:128*(tl+1)], rhs=sel[:],
                            start=True, stop=True)
                    zi = hw.tile([128, 8], F32, name="zi", tag="zi")
                    nc.vector.reciprocal(zi[:], psz)
                    for tl in range(4):
                        g0 = ck * 512 + tl * 128
                        bl, t0 = g0 // T, g0 % T
                        for h in range(2):
                            c = h * BPC + bl
                            psg = ps6.tile([128, SP], F32, name="p6",
                                           tag="p6")
                            nc.tensor.matmul(
                                psg,
                                lhsT=ut[64*h:64*(h+1), 128*tl:128*(tl+1)],
                                rhs=E2[64*h:64*(h+1), c*SP:(c+1)*SP],
                                start=True, stop=True)
                            stg = hw.tile([128, SP], F32, name="stg",
                                          tag="stg")
                            nc.vector.tensor_scalar(
                                stg[:], psg, zi[:, 2*tl+h:2*tl+h+1],
                                None, OP.mult)
                            # window w in [2, WIN) <-> stage col OWN*k + w
                            dst = phD_r[c*KSL:(c+1)*KSL, t0:t0+128, :]\
                                .rearrange("k t j -> t k j")
                            src_o = stg[:, HW:HW+KSL*OWN].rearrange(
                                "p (k j) -> p k j", j=OWN)
                            nc.sync.dma_start(
                                dst[:, :, HW-2:PJ], src_o)
                            src_h = stg[:, 2:2+KSL*OWN].rearrange(
                                "p (k j) -> p k j", j=OWN)[:, :, 0:HW-2]
                            nc.sync.dma_start(
                                dst[:, :, 0:HW-2], src_h)

            if stop_after <= 4:
                with tc.tile_pool(name="dum4", bufs=1) as dm:
                    td = dm.tile([128, 2], F32, name="td4")
                    nc.sync.dma_start(td[:, 0:1], phD[:, 0:1])
                    nc.sync.dma_start(td[:, 1:2], phD[:, T*PJ-1:T*PJ])
                    nc.sync.dma_start(outs['d_out'], td[:, 0:1])
                return

            # ---- phase 5: CTC alpha DP, packed layout with recomputed halo
            CH = 64                       # DP steps per streamed chunk
            with tc.tile_pool(name="pch", bufs=2) as pcp, \
                 tc.tile_pool(name="psD", bufs=2, space="PSUM") as psD, \
                 tc.tile_pool(name="dwork", bufs=1) as dw:
                alpha = dw.tile([128, WIN], F32, name="alpha")
                wt = dw.tile([128, WIN], F32, name="wt")
                vt = dw.tile([128, PJ], F32, name="vt")
                ratio = const.tile_from(ins['ratio0'], name="ratio_sb")
                msb = dw.tile([128, NR], F32, name="msb")
                minv = dw.tile([128, 1], F32, name="minv")
                mprev = dw.tile([128, 1], F32, name="mprev")
                d_sb = dw.tile([128, 1], F32, name="d_sb")
                nc.vector.memset(alpha[:], 0.0)
                nc.vector.memset(wt[:], 0.0)
                nc.vector.memset(msb[:], 1.0)

                pch = None
                for t in range(T):
                    if t % CH == 0:
                        pch = pcp.tile([128, PJ * CH], F32, name="pch",
                                       tag="pch")
                        nc.sync.dma_start(
                            pch[:], phD[:, t*PJ:(t+CH)*PJ])
                    pt = pch[:, (t % CH)*PJ:(t % CH + 1)*PJ]
                    if t == 0:
                        nc.vector.tensor_tensor(
                            alpha[:, 2:WIN], pt, initM[:], OP.mult)
                    else:
                        nc.vector.tensor_tensor(
                            vt[:], maskM[:], alpha[:, 0:PJ], OP.mult)
                        nc.vector.tensor_tensor(
                            wt[:, 2:WIN], alpha[:, 2:WIN],
                            alpha[:, 1:WIN-1], OP.add)
                        nc.vector.tensor_tensor(
                            wt[:, 2:WIN], wt[:, 2:WIN], vt[:], OP.add)
                        nc.vector.tensor_tensor(
                            alpha[:, 2:WIN], wt[:, 2:WIN], pt, OP.mult)
                    if t % RENORM == 0 and t > 0 and t < T - 1:
                        if True:
                            r = t // RENORM
                            # renorm down by per-partition max (guarded >= 1)
                            nc.vector.tensor_reduce(
                                msb[:, r:r+1], alpha[:, HW:WIN], AX.X, OP.max)
                            nc.vector.tensor_scalar(
                                msb[:, r:r+1], msb[:, r:r+1], 1.0, None,
                                OP.max)
                            nc.vector.reciprocal(minv[:], msb[:, r:r+1])
                            nc.vector.tensor_scalar(
                                alpha[:], alpha[:], minv[:], None, OP.mult)
                            # ratio[p] *= m'[p-1] * minv[p]  (shift via PE)
                            psm = psD.tile([128, 1], F32, name="psm",
                                           tag="psm")
                            nc.tensor.matmul(
                                psm, lhsT=P1[:], rhs=msb[:, r:r+1],
                                start=True, stop=True)
                            nc.vector.tensor_tensor(
                                mprev[:], ratio[:], psm, OP.mult)
                            nc.vector.tensor_tensor(
                                ratio[:], mprev[:], minv[:], OP.mult)
                        # refresh halo from left neighbour (shift via PE)
                        psh = psD.tile([128, HW], F32, name="psh", tag="psh")
                        nc.tensor.matmul(
                            psh, lhsT=P1[:], rhs=alpha[:, OWN:WIN],
                            start=True, stop=True)
                        nc.vector.tensor_scalar(
                            alpha[:, 0:HW], psh, ratio[:], None, OP.mult)

                nc.vector.scalar_tensor_tensor(
                    wt[:], alpha[:], 1.0, F_big[:], OP.mult, OP.mult,
                    accum_out=d_sb[:])
                nc.sync.dma_start(outs['d_out'], d_sb[:])
                nc.sync.dma_start(outs['ms_out'], msb[:])


# ----------------------------------------------------------------- interface

_CACHE = {}
_CACHE_DIR = "/tmp/brnnctc_kernel_cache_v1"


class _NcShim:
    """Minimal stand-in for a finalized Bacc object, backed by a BIR module
    deserialized from the on-disk cache (so HLO bytes are identical across
    processes and the jax persistent compilation cache can hit)."""

    def __init__(self, m, partition_name):
        import types as _types
        self.m = m
        self.dbg_addr = None
        self.dbg_callbacks = ()
        self.target_bir_lowering = False
        self.has_collectives = False
        self.trn_type = "TRN2"
        self.partition_id_tensor = (
            _types.SimpleNamespace(name=partition_name)
            if partition_name else None)

    def is_finalized(self):
        return True

    def to_json_bytes(self):
        return mybir.module_to_json_bytes(self.m)


def _get_program():
    if 'nc' in _CACHE:
        return _CACHE['nc']
    import os, json, zlib
    os.makedirs(_CACHE_DIR, exist_ok=True)
    bj = os.path.join(_CACHE_DIR, "bir.json.z")
    mj = os.path.join(_CACHE_DIR, "meta.json")
    if not (os.path.exists(bj) and os.path.exists(mj)):
        nc = build_program()
        pname = (nc.partition_id_tensor.name
                 if nc.partition_id_tensor is not None else None)
        blob = zlib.compress(mybir.module_to_json_bytes(nc.m), 1)
        tmp = bj + ".tmp"
        with open(tmp, "wb") as f:
            f.write(blob)
        os.replace(tmp, bj)
        with open(mj + ".tmp", "w") as f:
            json.dump({"partition": pname}, f)
        os.replace(mj + ".tmp", mj)
    with open(mj) as f:
        meta = json.load(f)
    with open(bj, "rb") as f:
        m = mybir.module_from_json_bytes(zlib.decompress(f.read()))
    shim = _NcShim(m, meta["partition"])
    _CACHE['nc'] = shim
    return shim


def _get_runner():
    if 'runner' in _CACHE:
        return _CACHE['runner']
    import os
    import jax
    from jax.sharding import Mesh, PartitionSpec
    from jax.experimental.shard_map import shard_map
    try:
        jax.config.update("jax_compilation_cache_dir",
                          os.path.join(_CACHE_DIR, "jaxcache"))
        jax.config.update("jax_persistent_cache_min_entry_size_bytes", -1)
        jax.config.update("jax_persistent_cache_min_compile_time_secs", 0)
    except Exception:
        pass
    from concourse.bass2jax import (
        _bass_exec_p, partition_id_tensor, install_neuronx_cc_hook)
    install_neuronx_cc_hook()
    nc = _get_program()
    partition_name = (nc.partition_id_tensor.name
                      if nc.partition_id_tensor else None)
    in_names, out_names, out_avals, zero_shapes = [], [], [], []
    for alloc in nc.m.functions[0].allocations:
        if not isinstance(alloc, mybir.MemoryLocationSet):
            continue
        name = alloc.memorylocations[0].name
        if alloc.kind == "ExternalInput":
            if name != partition_name:
                in_names.append(name)
        elif alloc.kind == "ExternalOutput":
            shape = tuple(alloc.tensor_shape)
            dtype = mybir.dt.np(alloc.dtype)
            out_names.append(name)
            out_avals.append(jax.core.ShapedArray(shape, dtype))
            zero_shapes.append((shape, dtype))
    n_params = len(in_names)
    all_names = list(in_names) + list(out_names)
    if partition_name is not None:
        all_names.append(partition_name)

    def _body(*args):
        operands = list(args)
        if partition_name is not None:
            operands.append(partition_id_tensor())
        outs = _bass_exec_p.bind(
            *operands, out_avals=tuple(out_avals),
            in_names=tuple(all_names), out_names=tuple(out_names),
            lowering_input_output_aliases=(), sim_require_finite=True,
            sim_require_nnan=True, nc=nc)
        return tuple(outs)

    devices = jax.devices()[:NCORES]
    mesh = Mesh(np.asarray(devices), ("core",))
    nin = n_params + len(zero_shapes)
    sharded = jax.jit(
        shard_map(_body, mesh=mesh,
                  in_specs=(PartitionSpec("core"),) * nin,
                  out_specs=(PartitionSpec("core"),) * len(out_names),
                  check_rep=False),
        keep_unused=True)
    zeros = [np.zeros((NCORES * s[0], *s[1:]), d) for s, d in zero_shapes]
    runner = (sharded, in_names, out_names, zeros)
    _CACHE['runner'] = runner
    return runner


def kernel(**inputs):
    assert np.all(np.asarray(inputs['inputs_length']) == T), \
        "kernel assumes full-length inputs"
    sharded, in_names, out_names, zeros = _get_runner()
    shared = _prep_shared(inputs)
    in_maps, tls_all = [], []
    for core in range(NCORES):
        m, tls = _prep_core(inputs, core, shared)
        in_maps.append(m)
        tls_all.append(tls)

    concat_in = [np.concatenate([in_maps[c][n] for c in range(NCORES)],
                                axis=0) for n in in_names]
    import jax as _jax
    import time as _time
    t0 = _time.time()
    out_arrs = sharded(*concat_in, *zeros)
    _jax.block_until_ready(out_arrs)
    _CACHE['last_exec_wall_ns'] = int((_time.time() - t0) * 1e9)

    class _Res:
        pass
    res = _Res()
    res.results = []
    for c in range(NCORES):
        dd = {}
        for i, name in enumerate(out_names):
            arr = np.asarray(out_arrs[i])
            rows = arr.shape[0] // NCORES
            dd[name] = arr[c*rows:(c+1)*rows]
        res.results.append(dd)

    base_losses, rle_losses = [], []
    for core in range(NCORES):
        d = np.float64(res.results[core]['d_out'][:, 0])
        ms = np.float64(res.results[core]['ms_out'])
        lam = np.sum(np.log(ms), axis=1)          # [128] log Lambda_p
        tls = tls_all[core]
        for h in range(2):
            for bl in range(BPC):
                c = h * BPC + bl
                dk = d[c*KSL:(c+1)*KSL]
                lk = lam[c*KSL:(c+1)*KSL]
                good = dk > 0
                if not np.any(good):
                    ll = -np.inf
                else:
                    terms = np.log(dk[good]) + lk[good]
                    mx = np.max(terms)
                    ll = mx + np.log(np.sum(np.exp(terms - mx)))
                ll -= T * np.log(C_SCALE)
                loss = -ll / tls[c]
                (base_losses if h == 0 else rle_losses).append(loss)
    return np.asarray(
        [np.mean(base_losses), np.mean(rle_losses)], np.float32)


def last_hw_time_ns():
    return _CACHE.get('last_exec_wall_ns')


# revision 42
# speedup vs baseline: 8.5532x; 1.0206x over previous
"""BRNN-CTC loss kernel on 8 NeuronCores via Bass/Tile.

Strategy: data-parallel over batch B=32 -> 4 sequences/core.
Device computes: input GEMMs, BiLSTM recurrence (fwd+bwd chains),
projection, two CTC heads (softmax via PE column sums), label-prob
gather via one-hot matmul, and the CTC alpha DP in normalized linear
space (packed layout: partition=(chain, S-slice), free=S-within-slice).
Host does: weight repacking, mask/one-hot building, and the final tiny
log-sum reductions (a few hundred floats).
"""
import numpy as np
import ml_dtypes
from contextlib import ExitStack

import concourse.bass as bass
import concourse.bacc as bacc
import concourse.mybir as mybir
import concourse.tile as tile
from concourse.tile_rust import add_dep_helper
from concourse import bass_utils

BF16 = mybir.dt.bfloat16
F32 = mybir.dt.float32
AX = mybir.AxisListType
OP = mybir.AluOpType
AF = mybir.ActivationFunctionType

NCORES = 8
T, B, F, H, INNER, V, L = 1024, 32, 128, 128, 512, 64, 200
S = 2 * L + 1            # 401
BPC = B // NCORES        # 4 sequences per core
NCH = 2 * BPC            # 8 CTC chains per core (2 heads x 4 seqs)
KSL = 16                 # S-slices per chain -> 8*16 = 128 partitions
OWN = 26                 # owned S positions per slice (16*26=416 >= 401)
HW = 16                  # left halo width (recomputed, refreshed every RENORM)
WIN = HW + OWN           # alpha window per partition (42)
PJ = WIN - 2             # p-hat cols per partition per step (40)
LPAD = HW                # left zero-pad in gather: col = s + LPAD
SP = LPAD + S + 15       # padded gather width (432)
C_SCALE = 64.0           # anti-drift constant folded into one-hot E
RENORM = 8               # renorm every 8 DP steps
NR = (T - 1) // RENORM + 2


def _np_bf16(x):
    return np.asarray(x, dtype=ml_dtypes.bfloat16)


# ----------------------------------------------------------------- host prep

def _pack_lstm_weights(Wih, Whh, b):
    """Reorder gate blocks to [i, f, o, 2*g] and transpose for lhsT."""
    def blocks(W):
        i, f, g, o = W[0:H], W[H:2*H], W[2*H:3*H], W[3*H:4*H]
        return np.concatenate([i, f, o, 2.0 * g], axis=0)
    Wihb, Whhb, bb = blocks(Wih), blocks(Whh), blocks(b.reshape(4*H, 1))[:, 0]
    wihT = np.concatenate([Wihb[128*g:128*(g+1)].T for g in range(4)], axis=1)
    whhT = np.concatenate([Whhb[128*g:128*(g+1)].T for g in range(4)], axis=1)
    bias = np.stack([bb[128*g:128*(g+1)] for g in range(4)], axis=1)
    return _np_bf16(wihT), _np_bf16(whhT), np.float32(bias)


def _build_ctc_host(tgt, tlen):
    """Per-chain ext labels / masks. tgt:[S-labels row], returns dicts."""
    ext = np.zeros(S, np.int64)
    ext[1::2] = tgt
    skip = np.zeros(S, np.float32)
    sr = np.arange(S)
    skip[(sr % 2 == 1) & (sr >= 2)] = 1.0
    skip[2:][ext[2:] == ext[:-2]] = 0.0
    fin = np.zeros(S, np.float32)
    fin[2 * tlen] = 1.0
    fin[2 * tlen - 1] = 1.0
    return ext, skip, fin


def _prep_core(inputs, core, shared):
    b0 = core * BPC
    x = np.asarray(inputs['inputs'][b0:b0 + BPC], np.float32)
    xT = _np_bf16(x.transpose(2, 0, 1).reshape(H, BPC * T))

    E2 = np.zeros((128, NCH * SP), np.float32)
    maskM = np.zeros((128, PJ), np.float32)
    F_big = np.zeros((128, WIN), np.float32)
    initM = np.zeros((128, PJ), np.float32)
    ratio0 = np.ones((128, 1), np.float32)
    tls = np.zeros(NCH, np.float32)
    for h in range(2):
        tgts = inputs['targets'] if h == 0 else inputs['rles']
        lens = inputs['targets_length'] if h == 0 else inputs['rles_length']
        for bl in range(BPC):
            c = h * BPC + bl
            ext, skip, fin = _build_ctc_host(
                np.asarray(tgts[b0 + bl], np.int64),
                int(lens[b0 + bl]))
            tls[c] = float(lens[b0 + bl])
            E = np.zeros((V, SP), np.float32)
            E[ext, LPAD + np.arange(S)] = C_SCALE
            E2[0:64, c * SP:(c + 1) * SP] = E
            E2[64:128, c * SP:(c + 1) * SP] = E
            for k in range(KSL):
                p = c * KSL + k
                # window position w covers s(w) = 26*k + w - HW, w in [0,WIN)
                for w in range(2, WIN):
                    s = OWN * k + w - HW
                    if 0 <= s < S:
                        maskM[p, w - 2] = skip[s]
                        if s <= 1:
                            initM[p, w - 2] = 1.0
                if k > 0 or True:
                    pass
                s0 = k * OWN
                n = min(OWN, S - s0)
                if n > 0:
                    F_big[p, HW:HW + n] = fin[s0:s0 + n]
            ratio0[c * KSL, 0] = 0.0

    m = dict(shared)
    m.update(xT=xT, E2=_np_bf16(E2), maskM=maskM, F_big=F_big,
             initM=initM, ratio0=ratio0)
    return m, tls


def _prep_shared(inputs):
    wihT_f, whhT_f, bias_f = _pack_lstm_weights(
        np.float32(inputs['W_ih_f']), np.float32(inputs['W_hh_f']),
        np.float32(inputs['b_f']))
    wihT_b, whhT_b, bias_b = _pack_lstm_weights(
        np.float32(inputs['W_ih_b']), np.float32(inputs['W_hh_b']),
        np.float32(inputs['b_b']))

    Wf2 = 2.0 * np.float32(inputs['W_fwd'])           # [INNER, 2H]
    wfT_k1 = np.concatenate(
        [Wf2[128*i:128*(i+1), 0:128].T for i in range(4)], axis=1)
    wfT_k2 = np.concatenate(
        [Wf2[128*i:128*(i+1), 128:256].T for i in range(4)], axis=1)
    bias_fw = np.stack(
        [2.0 * np.float32(inputs['b_fwd'])[128*i:128*(i+1)]
         for i in range(4)], axis=1)

    Wcat = np.concatenate(
        [np.float32(inputs['W_base']), np.float32(inputs['W_rle'])], axis=0)
    whT = np.concatenate(
        [Wcat[:, 128*k:128*(k+1)].T for k in range(4)], axis=1)
    bias_h = np.concatenate(
        [np.float32(inputs['b_base']), np.float32(inputs['b_rle'])]
    ).reshape(128, 1)

    ident = _np_bf16(np.eye(128, dtype=np.float32))
    sel = np.zeros((128, 2), np.float32)
    sel[0:64, 0] = 1.0
    sel[64:128, 1] = 1.0
    P1 = np.zeros((128, 128), np.float32)     # out[m] = in[m-1]
    P1[np.arange(127), np.arange(1, 128)] = 1.0

    return dict(
        wihT_f=wihT_f, whhT_f=whhT_f, bias_f=np.float32(bias_f),
        wihT_b=wihT_b, whhT_b=whhT_b, bias_b=np.float32(bias_b),
        wfT_k1=_np_bf16(wfT_k1), wfT_k2=_np_bf16(wfT_k2),
        bias_fw=np.float32(bias_fw),
        whT=_np_bf16(whT), bias_h=np.float32(bias_h),
        ident=ident, sel=_np_bf16(sel), P1=np.float32(P1))


# ------------------------------------------------------------- device kernel

IN_SPECS = [
    ('xT', (H, BPC * T), BF16),
    ('wihT_f', (128, 512), BF16), ('whhT_f', (128, 512), BF16),
    ('bias_f', (128, 4), F32),
    ('wihT_b', (128, 512), BF16), ('whhT_b', (128, 512), BF16),
    ('bias_b', (128, 4), F32),
    ('wfT_k1', (128, 512), BF16), ('wfT_k2', (128, 512), BF16),
    ('bias_fw', (128, 4), F32),
    ('whT', (128, 512), BF16), ('bias_h', (128, 1), F32),
    ('ident', (128, 128), BF16), ('sel', (128, 2), BF16),
    ('P1', (128, 128), F32),
    ('E2', (128, NCH * SP), BF16),
    ('maskM', (128, PJ), F32), ('F_big', (128, WIN), F32),
    ('initM', (128, PJ), F32), ('ratio0', (128, 1), F32),
]
OUT_SPECS = [
    ('d_out', (128, 1), F32),
    ('ms_out', (128, NR), F32),
]


def build_program(stop_after=99):
    nc = bacc.Bacc(
        "TRN2", target_bir_lowering=False, debug=False,
        enable_asserts=False, num_devices=NCORES)
    ins = {n: nc.dram_tensor(n, list(s), d, kind="ExternalInput").ap()
           for n, s, d in IN_SPECS}
    outs = {n: nc.dram_tensor(n, list(s), d, kind="ExternalOutput").ap()
            for n, s, d in OUT_SPECS}
    with tile.TileContext(nc) as tc:
        _emit(tc, outs, ins, stop_after)
    nc.finalize()
    return nc


def _emit(tc, outs, ins, stop_after=99):
    nc = tc.nc
    NT = BPC * T               # total (b, t) columns
    NCK = NT // 512            # 512-col chunks
    with ExitStack() as ctx:
        const = ctx.enter_context(tc.tile_pool(name="const", bufs=1))

        def load(name, dtype=None):
            return const.tile_from(ins[name], name=name + "_sb", dtype=dtype)

        xT = load('xT')
        wihT = {0: load('wihT_f'), 1: load('wihT_b')}
        whhT = {0: load('whhT_f'), 1: load('whhT_b')}
        bias = {0: load('bias_f'), 1: load('bias_b')}
        wfT = {0: load('wfT_k1'), 1: load('wfT_k2')}
        bias_fw = load('bias_fw')
        whT = load('whT')
        bias_h = load('bias_h')
        ident = load('ident')
        sel = load('sel')
        P1 = load('P1')
        E2 = load('E2')
        maskM = load('maskM')
        F_big = load('F_big')
        initM = load('initM')

        # big SBUF state
        # xw col = g*(2*NT) + d*NT + b*T + t   (g in [i,f,o,2g])
        xw = const.tile([128, 8 * NT], BF16, name="xw")
        # h col = d*NT + b*T + t
        hst = const.tile([128, 2 * NT], BF16, name="hst")
        fo = [const.tile([128, NT], BF16, name=f"fo{i}") for i in range(4)]

        # ---- phase 1: xW = Wih @ x (+bias), bf16
        with tc.tile_pool(name="ps1", bufs=4, space="PSUM") as ps1:
            for d in range(2):
                for g in range(4):
                    for ck in range(NCK):
                        ps = ps1.tile([128, 512], F32, name="p1")
                        nc.tensor.matmul(
                            ps, lhsT=wihT[d][:, 128*g:128*(g+1)],
                            rhs=xT[:, 512*ck:512*(ck+1)],
                            start=True, stop=True)
                        c0 = (2*g + d) * NT + 512*ck
                        nc.vector.tensor_scalar(
                            xw[:, c0:c0+512],
                            ps, bias[d][:, g:g+1], None, OP.add)

        def _dummy_out(*src_aps):
            with tc.tile_pool(name="dumm", bufs=1) as dm:
                for i, ap in enumerate(src_aps):
                    tdum = dm.tile([128, 1], F32, name=f"tdum{i}")
                    nc.vector.tensor_copy(tdum[:], ap)
                    dst = outs['d_out'] if i == 0 else outs['ms_out'][:, i-1:i]
                    nc.sync.dma_start(dst, tdum[:])

        if stop_after <= 1:
            _dummy_out(xw[:, 0:1], xw[:, 8*NT-1:8*NT])
            return

        # ---- phase 2: BiLSTM recurrence. Two independent chains (fwd d=0,
        # bwd d=1) emitted on a hand-placed modulo schedule (half-period
        # offset) so their engine bursts interleave instead of colliding.
        # xw col = g*(2*NT) + d*NT + b*T + t; psum col (per dir) = g*4 + b.
        xw_r = xw.rearrange("p (g d b t) -> p g d b t", g=4, d=2, b=BPC)
        h_r = hst.rearrange("p (d b t) -> p d b t", d=2, b=BPC)
        with tc.tile_pool(name="ps2", bufs=3, space="PSUM") as ps2, \
             tc.tile_pool(name="lwork", bufs=4) as lw:
            cst = {d: const.tile([128, BPC], F32, name=f"c{d}")
                   for d in range(2)}
            for d in range(2):
                nc.vector.memset(cst[d][:], 0.0)

            psums, ghats, shs = {}, {}, {}
            _last = {}

            def _ord(key, binst):
                # force per-engine issue order to match emission order
                prev = _last.get(key)
                if prev is not None:
                    add_dep_helper(binst.ins, prev.ins, sync=False,
                                   reason="modulo-order")
                _last[key] = binst
                return binst

            def part_mm(d, i):
                t = i if d == 0 else T - 1 - i
                tp = t - 1 if d == 0 else t + 1
                ps = ps2.tile([128, 16], F32, name=f"psg{d}", tag=f"psg{d}")
                _ord("PE", nc.tensor.matmul(
                    ps, lhsT=ident[:], rhs=xw_r[:, :, d, :, t],
                    start=True, stop=(i == 0)))
                if i > 0:
                    for g in range(4):
                        _ord("PE", nc.tensor.matmul(
                            ps[:, 4*g:4*(g+1)],
                            lhsT=whhT[d][:, 128*g:128*(g+1)],
                            rhs=h_r[:, d, :, tp],
                            start=False, stop=(g == 3)))
                psums[d] = ps

            def part_sig(d, i):
                ghat = lw.tile([128, 16], F32, name=f"ghat{d}",
                               tag=f"ghat{d}")
                _ord("ACT", nc.scalar.activation(ghat[:], psums[d],
                                                 AF.Sigmoid))
                ghats[d] = ghat

            def part_cpath(d, i):
                ghat = ghats[d]
                u = lw.tile([128, 4], F32, name=f"u{d}", tag=f"u{d}")
                _ord("DVE", nc.vector.tensor_tensor(
                    u[:], ghat[:, 4:8], cst[d][:], OP.mult))
                gt = lw.tile([128, 4], F32, name=f"gt{d}", tag=f"gt{d}")
                _ord("DVE", nc.vector.tensor_scalar(
                    gt[:], ghat[:, 12:16], 2.0, -1.0, OP.mult, OP.add))
                t1 = lw.tile([128, 4], F32, name=f"t1{d}", tag=f"t1{d}")
                _ord("DVE", nc.vector.tensor_tensor(
                    t1[:], ghat[:, 0:4], gt[:], OP.mult))
                _ord("DVE", nc.vector.tensor_tensor(
                    cst[d][:], u[:], t1[:], OP.add))

            def part_sigc(d, i):
                sh = lw.tile([128, 4], F32, name=f"sh{d}", tag=f"sh{d}")
                _ord("ACT", nc.scalar.activation(
                    sh[:], cst[d][:], AF.Tanh))
                shs[d] = sh

            def part_hpath(d, i):
                t = i if d == 0 else T - 1 - i
                _ord("DVE", nc.vector.tensor_tensor(
                    h_r[:, 0 if d == 0 else 1, :, t],
                    ghats[d][:, 8:12], shs[d][:], OP.mult))

            # modulo schedule, one period per step index i (phi order):
            # MM_f(0) hpath_b(135) MM_b(350) sig_f(520) cpath_f(745)
            # sig_b(870) cpath_b(1095) sigc_f(1230) hpath_f(1485) sigc_b(1580)
            part_mm(0, 0)
            part_mm(1, 0)
            part_sig(0, 0)
            part_cpath(0, 0)
            part_sig(1, 0)
            part_cpath(1, 0)
            part_sigc(0, 0)
            part_hpath(0, 0)
            part_sigc(1, 0)
            for i in range(1, T):
                part_mm(0, i)
                part_hpath(1, i - 1)
                part_mm(1, i)
                part_sig(0, i)
                part_cpath(0, i)
                part_sig(1, i)
                part_cpath(1, i)
                part_sigc(0, i)
                part_hpath(0, i)
                part_sigc(1, i)
            part_hpath(1, T - 1)

        if stop_after <= 2:
            _dummy_out(hst[:, NT-1:NT], hst[:, NT:NT+1])
            return

        # ---- phase 3: fo = tanh(Wf enc + b) via 2*sigmoid(2x)-1
        with tc.tile_pool(name="ps3", bufs=4, space="PSUM") as ps3, \
             tc.tile_pool(name="pwork", bufs=3) as pw:
            for ck in range(NCK):
                for isl in range(4):
                    ps = ps3.tile([128, 512], F32, name="p3")
                    nc.tensor.matmul(
                        ps, lhsT=wfT[0][:, 128*isl:128*(isl+1)],
                        rhs=hst[:, 512*ck:512*(ck+1)],
                        start=True, stop=False)
                    nc.tensor.matmul(
                        ps, lhsT=wfT[1][:, 128*isl:128*(isl+1)],
                        rhs=hst[:, NT + 512*ck:NT + 512*(ck+1)],
                        start=False, stop=True)
                    sp_ = pw.tile([128, 512], F32, name="sp", tag="sp")
                    nc.scalar.activation(
                        sp_[:], ps, AF.Sigmoid, bias=bias_fw[:, isl:isl+1])
                    nc.vector.tensor_scalar(
                        fo[isl][:, 512*ck:512*(ck+1)],
                        sp_[:], 2.0, -1.0, OP.mult, OP.add)

        if stop_after <= 3:
            _dummy_out(fo[0][:, 0:1], fo[3][:, NT-1:NT])
            return

        # table-set switch fence (sigmoid set -> exp set)
        tc.strict_bb_all_engine_barrier()

        # ---- phase 4: heads, softmax pieces, gather, relayout to DRAM
        with tc.tile_pool(name="dram", bufs=1, space="DRAM") as dp:
            phD = dp.tile([128, T * PJ], F32, name="phD")
            phD_r = phD.rearrange("r (t j) -> r t j", j=PJ)
            with tc.tile_pool(name="ps4", bufs=2, space="PSUM") as ps4, \
                 tc.tile_pool(name="ps5", bufs=2, space="PSUM") as ps5, \
                 tc.tile_pool(name="ps6", bufs=2, space="PSUM") as ps6, \
                 tc.tile_pool(name="hwork", bufs=2) as hw:
                for ck in range(NCK):
                    psl = ps4.tile([128, 512], F32, name="p4")
                    for k in range(4):
                        nc.tensor.matmul(
                            psl, lhsT=whT[:, 128*k:128*(k+1)],
                            rhs=fo[k][:, 512*ck:512*(ck+1)],
                            start=(k == 0), stop=(k == 3))
                    ut = hw.tile([128, 512], BF16, name="ut", tag="ut")
                    nc.scalar.activation(ut[:], psl, AF.Exp, bias=bias_h[:])
                    psz = ps5.tile([128, 8], F32, name="p5", tag="p5")
                    for tl in range(4):
                        nc.tensor.matmul(
                            psz[:, 2*tl:2*(tl+1)],
                            lhsT=ut[:, 128*tl:128*(tl+1)], rhs=sel[:],
                            start=True, stop=True)
                    zi = hw.tile([128, 8], F32, name="zi", tag="zi")
                    nc.vector.reciprocal(zi[:], psz)
                    for tl in range(4):
                        g0 = ck * 512 + tl * 128
                        bl, t0 = g0 // T, g0 % T
                        for h in range(2):
                            c = h * BPC + bl
                            psg = ps6.tile([128, SP], F32, name="p6",
                                           tag="p6")
                            nc.tensor.matmul(
                                psg,
                                lhsT=ut[64*h:64*(h+1), 128*tl:128*(tl+1)],
                                rhs=E2[64*h:64*(h+1), c*SP:(c+1)*SP],
                                start=True, stop=True)
                            stg = hw.tile([128, SP], F32, name="stg",
                                          tag="stg")
                            nc.vector.tensor_scalar(
                                stg[:], psg, zi[:, 2*tl+h:2*tl+h+1],
                                None, OP.mult)
                            # window w in [2, WIN) <-> stage col OWN*k + w
                            dst = phD_r[c*KSL:(c+1)*KSL, t0:t0+128, :]\
                                .rearrange("k t j -> t k j")
                            src_o = stg[:, HW:HW+KSL*OWN].rearrange(
                                "p (k j) -> p k j", j=OWN)
                            nc.sync.dma_start(
                                dst[:, :, HW-2:PJ], src_o)
                            src_h = stg[:, 2:2+KSL*OWN].rearrange(
                                "p (k j) -> p k j", j=OWN)[:, :, 0:HW-2]
                            nc.sync.dma_start(
                                dst[:, :, 0:HW-2], src_h)

            if stop_after <= 4:
                with tc.tile_pool(name="dum4", bufs=1) as dm:
                    td = dm.tile([128, 2], F32, name="td4")
                    nc.sync.dma_start(td[:, 0:1], phD[:, 0:1])
                    nc.sync.dma_start(td[:, 1:2], phD[:, T*PJ-1:T*PJ])
                    nc.sync.dma_start(outs['d_out'], td[:, 0:1])
                return

            # ---- phase 5: CTC alpha DP, packed layout with recomputed halo
            CH = 64                       # DP steps per streamed chunk
            with tc.tile_pool(name="pch", bufs=2) as pcp, \
                 tc.tile_pool(name="psD", bufs=2, space="PSUM") as psD, \
                 tc.tile_pool(name="dwork", bufs=1) as dw:
                alpha = dw.tile([128, WIN], F32, name="alpha")
                wt = dw.tile([128, WIN], F32, name="wt")
                vt = dw.tile([128, PJ], F32, name="vt")
                ratio = const.tile_from(ins['ratio0'], name="ratio_sb")
                msb = dw.tile([128, NR], F32, name="msb")
                minv = dw.tile([128, 1], F32, name="minv")
                mprev = dw.tile([128, 1], F32, name="mprev")
                d_sb = dw.tile([128, 1], F32, name="d_sb")
                nc.vector.memset(alpha[:], 0.0)
                nc.vector.memset(wt[:], 0.0)
                nc.vector.memset(msb[:], 1.0)

                pch = None
                for t in range(T):
                    if t % CH == 0:
                        pch = pcp.tile([128, PJ * CH], F32, name="pch",
                                       tag="pch")
                        nc.sync.dma_start(
                            pch[:], phD[:, t*PJ:(t+CH)*PJ])
                    pt = pch[:, (t % CH)*PJ:(t % CH + 1)*PJ]
                    if t == 0:
                        nc.vector.tensor_tensor(
                            alpha[:, 2:WIN], pt, initM[:], OP.mult)
                    else:
                        nc.vector.tensor_tensor(
                            vt[:], maskM[:], alpha[:, 0:PJ], OP.mult)
                        nc.vector.tensor_tensor(
                            wt[:, 2:WIN], alpha[:, 2:WIN],
                            alpha[:, 1:WIN-1], OP.add)
                        nc.vector.tensor_tensor(
                            wt[:, 2:WIN], wt[:, 2:WIN], vt[:], OP.add)
                        nc.vector.tensor_tensor(
                            alpha[:, 2:WIN], wt[:, 2:WIN], pt, OP.mult)
                    if t % RENORM == 0 and t > 0 and t < T - 1:
                        if True:
                            r = t // RENORM
                            # renorm down by per-partition max (guarded >= 1)
                            nc.vector.tensor_reduce(
                                msb[:, r:r+1], alpha[:, HW:WIN], AX.X, OP.max)
                            nc.vector.tensor_scalar(
                                msb[:, r:r+1], msb[:, r:r+1], 1.0, None,
                                OP.max)
                            nc.vector.reciprocal(minv[:], msb[:, r:r+1])
                            nc.vector.tensor_scalar(
                                alpha[:], alpha[:], minv[:], None, OP.mult)
                            # ratio[p] *= m'[p-1] * minv[p]  (shift via PE)
                            psm = psD.tile([128, 1], F32, name="psm",
                                           tag="psm")
                            nc.tensor.matmul(
                                psm, lhsT=P1[:], rhs=msb[:, r:r+1],
                                start=True, stop=True)
                            nc.vector.tensor_tensor(
                                mprev[:], ratio[:], psm, OP.mult)
                            nc.vector.tensor_tensor(
                                ratio[:], mprev[:], minv[:], OP.mult)
                        # refresh halo from left neighbour (shift via PE)
                        psh = psD.tile([128, HW], F32, name="psh", tag="psh")
                        nc.tensor.matmul(
                            psh, lhsT=P1[:], rhs=alpha[:, OWN:WIN],
                            start=True, stop=True)
                        nc.vector.tensor_scalar(
                            alpha[:, 0:HW], psh, ratio[:], None, OP.mult)

                nc.vector.scalar_tensor_tensor(
                    wt[:], alpha[:], 1.0, F_big[:], OP.mult, OP.mult,
                    accum_out=d_sb[:])
                nc.sync.dma_start(outs['d_out'], d_sb[:])
                nc.sync.dma_start(outs['ms_out'], msb[:])


# ----------------------------------------------------------------- interface

_CACHE = {}
_CACHE_DIR = "/tmp/brnnctc_kernel_cache_v1"


class _NcShim:
    """Minimal stand-in for a finalized Bacc object, backed by a BIR module
    deserialized from the on-disk cache (so HLO bytes are identical across
    processes and the jax persistent compilation cache can hit)."""

    def __init__(self, m, partition_name):
        import types as _types
        self.m = m
        self.dbg_addr = None
        self.dbg_callbacks = ()
        self.target_bir_lowering = False
        self.has_collectives = False
        self.trn_type = "TRN2"
        self.partition_id_tensor = (
            _types.SimpleNamespace(name=partition_name)
            if partition_name else None)

    def is_finalized(self):
        return True

    def to_json_bytes(self):
        return mybir.module_to_json_bytes(self.m)


def _get_program():
    if 'nc' in _CACHE:
        return _CACHE['nc']
    import os, json, zlib
    os.makedirs(_CACHE_DIR, exist_ok=True)
    bj = os.path.join(_CACHE_DIR, "bir.json.z")
    mj = os.path.join(_CACHE_DIR, "meta.json")
    if not (os.path.exists(bj) and os.path.exists(mj)):
        nc = build_program()
        pname = (nc.partition_id_tensor.name
                 if nc.partition_id_tensor is not None else None)
        blob = zlib.compress(mybir.module_to_json_bytes(nc.m), 1)
        tmp = bj + ".tmp"
        with open(tmp, "wb") as f:
            f.write(blob)
        os.replace(tmp, bj)
        with open(mj + ".tmp", "w") as f:
            json.dump({"partition": pname}, f)
        os.replace(mj + ".tmp", mj)
    with open(mj) as f:
        meta = json.load(f)
    with open(bj, "rb") as f:
        m = mybir.module_from_json_bytes(zlib.decompress(f.read()))
    shim = _NcShim(m, meta["partition"])
    _CACHE['nc'] = shim
    return shim


def _get_runner():
    if 'runner' in _CACHE:
        return _CACHE['runner']
    import os
    import jax
    from jax.sharding import Mesh, PartitionSpec
    from jax.experimental.shard_map import shard_map
    try:
        jax.config.update("jax_compilation_cache_dir",
                          os.path.join(_CACHE_DIR, "jaxcache"))
        jax.config.update("jax_persistent_cache_min_entry_size_bytes", -1)
        jax.config.update("jax_persistent_cache_min_compile_time_secs", 0)
    except Exception:
        pass
    from concourse.bass2jax import (
        _bass_exec_p, partition_id_tensor, install_neuronx_cc_hook)
    install_neuronx_cc_hook()
    nc = _get_program()
    partition_name = (nc.partition_id_tensor.name
                      if nc.partition_id_tensor else None)
    in_names, out_names, out_avals, zero_shapes = [], [], [], []
    for alloc in nc.m.functions[0].allocations:
        if not isinstance(alloc, mybir.MemoryLocationSet):
            continue
        name = alloc.memorylocations[0].name
        if alloc.kind == "ExternalInput":
            if name != partition_name:
                in_names.append(name)
        elif alloc.kind == "ExternalOutput":
            shape = tuple(alloc.tensor_shape)
            dtype = mybir.dt.np(alloc.dtype)
            out_names.append(name)
            out_avals.append(jax.core.ShapedArray(shape, dtype))
            zero_shapes.append((shape, dtype))
    n_params = len(in_names)
    all_names = list(in_names) + list(out_names)
    if partition_name is not None:
        all_names.append(partition_name)

    def _body(*args):
        operands = list(args)
        if partition_name is not None:
            operands.append(partition_id_tensor())
        outs = _bass_exec_p.bind(
            *operands, out_avals=tuple(out_avals),
            in_names=tuple(all_names), out_names=tuple(out_names),
            lowering_input_output_aliases=(), sim_require_finite=True,
            sim_require_nnan=True, nc=nc)
        return tuple(outs)

    devices = jax.devices()[:NCORES]
    mesh = Mesh(np.asarray(devices), ("core",))
    nin = n_params + len(zero_shapes)
    sharded = jax.jit(
        shard_map(_body, mesh=mesh,
                  in_specs=(PartitionSpec("core"),) * nin,
                  out_specs=(PartitionSpec("core"),) * len(out_names),
                  check_rep=False),
        keep_unused=True)
    zeros = [np.zeros((NCORES * s[0], *s[1:]), d) for s, d in zero_shapes]
    runner = (sharded, in_names, out_names, zeros)
    _CACHE['mesh'] = mesh
    _CACHE['runner'] = runner
    return runner


def kernel(**inputs):
    assert np.all(np.asarray(inputs['inputs_length']) == T), \
        "kernel assumes full-length inputs"
    sharded, in_names, out_names, zeros = _get_runner()
    shared = _prep_shared(inputs)
    in_maps, tls_all = [], []
    for core in range(NCORES):
        m, tls = _prep_core(inputs, core, shared)
        in_maps.append(m)
        tls_all.append(tls)

    concat_in = [np.concatenate([in_maps[c][n] for c in range(NCORES)],
                                axis=0) for n in in_names]
    import jax as _jax
    import time as _time
    t0 = _time.time()
    out_arrs = sharded(*concat_in, *zeros)
    _jax.block_until_ready(out_arrs)
    _CACHE['last_exec_wall_ns'] = int((_time.time() - t0) * 1e9)

    class _Res:
        pass
    res = _Res()
    res.results = []
    for c in range(NCORES):
        dd = {}
        for i, name in enumerate(out_names):
            arr = np.asarray(out_arrs[i])
            rows = arr.shape[0] // NCORES
            dd[name] = arr[c*rows:(c+1)*rows]
        res.results.append(dd)

    base_losses, rle_losses = [], []
    for core in range(NCORES):
        d = np.float64(res.results[core]['d_out'][:, 0])
        ms = np.float64(res.results[core]['ms_out'])
        lam = np.sum(np.log(ms), axis=1)          # [128] log Lambda_p
        tls = tls_all[core]
        for h in range(2):
            for bl in range(BPC):
                c = h * BPC + bl
                dk = d[c*KSL:(c+1)*KSL]
                lk = lam[c*KSL:(c+1)*KSL]
                good = dk > 0
                if not np.any(good):
                    ll = -np.inf
                else:
                    terms = np.log(dk[good]) + lk[good]
                    mx = np.max(terms)
                    ll = mx + np.log(np.sum(np.exp(terms - mx)))
                ll -= T * np.log(C_SCALE)
                loss = -ll / tls[c]
                (base_losses if h == 0 else rle_losses).append(loss)
    return np.asarray(
        [np.mean(base_losses), np.mean(rle_losses)], np.float32)


def last_hw_time_ns():
    return _CACHE.get('last_exec_wall_ns')


# revision 45
# speedup vs baseline: 9.0165x; 1.0542x over previous
"""BRNN-CTC loss kernel on 8 NeuronCores via Bass/Tile.

Strategy: data-parallel over batch B=32 -> 4 sequences/core.
Device computes: input GEMMs, BiLSTM recurrence (fwd+bwd chains),
projection, two CTC heads (softmax via PE column sums), label-prob
gather via one-hot matmul, and the CTC alpha DP in normalized linear
space (packed layout: partition=(chain, S-slice), free=S-within-slice).
Host does: weight repacking, mask/one-hot building, and the final tiny
log-sum reductions (a few hundred floats).
"""
import numpy as np
import ml_dtypes
from contextlib import ExitStack

import concourse.bass as bass
import concourse.bacc as bacc
import concourse.mybir as mybir
import concourse.tile as tile
from concourse.tile_rust import add_dep_helper
from concourse import bass_utils

BF16 = mybir.dt.bfloat16
F32 = mybir.dt.float32
AX = mybir.AxisListType
OP = mybir.AluOpType
AF = mybir.ActivationFunctionType

NCORES = 8
T, B, F, H, INNER, V, L = 1024, 32, 128, 128, 512, 64, 200
S = 2 * L + 1            # 401
BPC = B // NCORES        # 4 sequences per core
NCH = 2 * BPC            # 8 CTC chains per core (2 heads x 4 seqs)
KSL = 16                 # S-slices per chain -> 8*16 = 128 partitions
OWN = 26                 # owned S positions per slice (16*26=416 >= 401)
HW = 16                  # left halo width (recomputed, refreshed every RENORM)
WIN = HW + OWN           # alpha window per partition (42)
PJ = WIN - 2             # p-hat cols per partition per step (40)
LPAD = HW                # left zero-pad in gather: col = s + LPAD
SP = LPAD + S + 15       # padded gather width (432)
C_SCALE = 64.0           # anti-drift constant folded into one-hot E
RENORM = 8               # renorm every 8 DP steps
NR = (T - 1) // RENORM + 2


def _np_bf16(x):
    return np.asarray(x, dtype=ml_dtypes.bfloat16)


# ----------------------------------------------------------------- host prep

def _pack_lstm_weights(Wih, Whh, b):
    """Reorder gate blocks to [i, f, o, 2*g] and transpose for lhsT."""
    def blocks(W):
        i, f, g, o = W[0:H], W[H:2*H], W[2*H:3*H], W[3*H:4*H]
        return np.concatenate([i, f, o, 2.0 * g], axis=0)
    Wihb, Whhb, bb = blocks(Wih), blocks(Whh), blocks(b.reshape(4*H, 1))[:, 0]
    wihT = np.concatenate([Wihb[128*g:128*(g+1)].T for g in range(4)], axis=1)
    whhT = np.concatenate([Whhb[128*g:128*(g+1)].T for g in range(4)], axis=1)
    bias = np.stack([bb[128*g:128*(g+1)] for g in range(4)], axis=1)
    return _np_bf16(wihT), _np_bf16(whhT), np.float32(bias)


def _build_ctc_host(tgt, tlen):
    """Per-chain ext labels / masks. tgt:[S-labels row], returns dicts."""
    ext = np.zeros(S, np.int64)
    ext[1::2] = tgt
    skip = np.zeros(S, np.float32)
    sr = np.arange(S)
    skip[(sr % 2 == 1) & (sr >= 2)] = 1.0
    skip[2:][ext[2:] == ext[:-2]] = 0.0
    fin = np.zeros(S, np.float32)
    fin[2 * tlen] = 1.0
    fin[2 * tlen - 1] = 1.0
    return ext, skip, fin


def _prep_core(inputs, core, shared):
    b0 = core * BPC
    x = np.asarray(inputs['inputs'][b0:b0 + BPC], np.float32)
    xT = _np_bf16(x.transpose(2, 0, 1).reshape(H, BPC * T))

    E2 = np.zeros((128, NCH * SP), np.float32)
    maskM = np.zeros((128, PJ), np.float32)
    F_big = np.zeros((128, WIN), np.float32)
    initM = np.zeros((128, PJ), np.float32)
    ratio0 = np.ones((128, 1), np.float32)
    tls = np.zeros(NCH, np.float32)
    for h in range(2):
        tgts = inputs['targets'] if h == 0 else inputs['rles']
        lens = inputs['targets_length'] if h == 0 else inputs['rles_length']
        for bl in range(BPC):
            c = h * BPC + bl
            ext, skip, fin = _build_ctc_host(
                np.asarray(tgts[b0 + bl], np.int64),
                int(lens[b0 + bl]))
            tls[c] = float(lens[b0 + bl])
            E = np.zeros((V, SP), np.float32)
            E[ext, LPAD + np.arange(S)] = C_SCALE
            E2[0:64, c * SP:(c + 1) * SP] = E
            E2[64:128, c * SP:(c + 1) * SP] = E
            for k in range(KSL):
                p = c * KSL + k
                # window position w covers s(w) = 26*k + w - HW, w in [0,WIN)
                for w in range(2, WIN):
                    s = OWN * k + w - HW
                    if 0 <= s < S:
                        maskM[p, w - 2] = skip[s]
                        if s <= 1:
                            initM[p, w - 2] = 1.0
                if k > 0 or True:
                    pass
                s0 = k * OWN
                n = min(OWN, S - s0)
                if n > 0:
                    F_big[p, HW:HW + n] = fin[s0:s0 + n]
            ratio0[c * KSL, 0] = 0.0

    m = dict(shared)
    m.update(xT=xT, E2=_np_bf16(E2), maskM=maskM, F_big=F_big,
             initM=initM, ratio0=ratio0)
    return m, tls


def _prep_shared(inputs):
    wihT_f, whhT_f, bias_f = _pack_lstm_weights(
        np.float32(inputs['W_ih_f']), np.float32(inputs['W_hh_f']),
        np.float32(inputs['b_f']))
    wihT_b, whhT_b, bias_b = _pack_lstm_weights(
        np.float32(inputs['W_ih_b']), np.float32(inputs['W_hh_b']),
        np.float32(inputs['b_b']))

    Wf2 = 2.0 * np.float32(inputs['W_fwd'])           # [INNER, 2H]
    wfT_k1 = np.concatenate(
        [Wf2[128*i:128*(i+1), 0:128].T for i in range(4)], axis=1)
    wfT_k2 = np.concatenate(
        [Wf2[128*i:128*(i+1), 128:256].T for i in range(4)], axis=1)
    bias_fw = np.stack(
        [2.0 * np.float32(inputs['b_fwd'])[128*i:128*(i+1)]
         for i in range(4)], axis=1)

    Wcat = np.concatenate(
        [np.float32(inputs['W_base']), np.float32(inputs['W_rle'])], axis=0)
    whT = np.concatenate(
        [Wcat[:, 128*k:128*(k+1)].T for k in range(4)], axis=1)
    bias_h = np.concatenate(
        [np.float32(inputs['b_base']), np.float32(inputs['b_rle'])]
    ).reshape(128, 1)

    ident = _np_bf16(np.eye(128, dtype=np.float32))
    sel = np.zeros((128, 2), np.float32)
    sel[0:64, 0] = 1.0
    sel[64:128, 1] = 1.0
    P1 = np.zeros((128, 128), np.float32)     # out[m] = in[m-1]
    P1[np.arange(127), np.arange(1, 128)] = 1.0

    return dict(
        wihT_f=wihT_f, whhT_f=whhT_f, bias_f=np.float32(bias_f),
        wihT_b=wihT_b, whhT_b=whhT_b, bias_b=np.float32(bias_b),
        wfT_k1=_np_bf16(wfT_k1), wfT_k2=_np_bf16(wfT_k2),
        bias_fw=np.float32(bias_fw),
        whT=_np_bf16(whT), bias_h=np.float32(bias_h),
        ident=ident, sel=_np_bf16(sel), P1=np.float32(P1))


# ------------------------------------------------------------- device kernel

IN_SPECS = [
    ('xT', (H, BPC * T), BF16),
    ('wihT_f', (128, 512), BF16), ('whhT_f', (128, 512), BF16),
    ('bias_f', (128, 4), F32),
    ('wihT_b', (128, 512), BF16), ('whhT_b', (128, 512), BF16),
    ('bias_b', (128, 4), F32),
    ('wfT_k1', (128, 512), BF16), ('wfT_k2', (128, 512), BF16),
    ('bias_fw', (128, 4), F32),
    ('whT', (128, 512), BF16), ('bias_h', (128, 1), F32),
    ('ident', (128, 128), BF16), ('sel', (128, 2), BF16),
    ('P1', (128, 128), F32),
    ('E2', (128, NCH * SP), BF16),
    ('maskM', (128, PJ), F32), ('F_big', (128, WIN), F32),
    ('initM', (128, PJ), F32), ('ratio0', (128, 1), F32),
]
OUT_SPECS = [
    ('d_out', (128, 1), F32),
    ('ms_out', (128, NR), F32),
]


def build_program(stop_after=99):
    nc = bacc.Bacc(
        "TRN2", target_bir_lowering=False, debug=False,
        enable_asserts=False, num_devices=NCORES)
    ins = {n: nc.dram_tensor(n, list(s), d, kind="ExternalInput").ap()
           for n, s, d in IN_SPECS}
    outs = {n: nc.dram_tensor(n, list(s), d, kind="ExternalOutput").ap()
            for n, s, d in OUT_SPECS}
    with tile.TileContext(nc) as tc:
        _emit(tc, outs, ins, stop_after)
    nc.finalize()
    return nc


def _emit(tc, outs, ins, stop_after=99):
    nc = tc.nc
    NT = BPC * T               # total (b, t) columns
    NCK = NT // 512            # 512-col chunks
    with ExitStack() as ctx:
        const = ctx.enter_context(tc.tile_pool(name="const", bufs=1))

        def load(name, dtype=None):
            return const.tile_from(ins[name], name=name + "_sb", dtype=dtype)

        xT = load('xT')
        wihT = {0: load('wihT_f'), 1: load('wihT_b')}
        whhT = {0: load('whhT_f'), 1: load('whhT_b')}
        bias = {0: load('bias_f'), 1: load('bias_b')}
        wfT = {0: load('wfT_k1'), 1: load('wfT_k2')}
        bias_fw = load('bias_fw')
        whT = load('whT')
        bias_h = load('bias_h')
        ident = load('ident')
        sel = load('sel')
        P1 = load('P1')
        E2 = load('E2')
        maskM = load('maskM')
        F_big = load('F_big')
        initM = load('initM')

        # big SBUF state
        # xw col = g*(2*NT) + d*NT + b*T + t   (g in [i,f,o,2g])
        xw = const.tile([128, 8 * NT], BF16, name="xw")
        # h col = d*NT + b*T + t
        hst = const.tile([128, 2 * NT], BF16, name="hst")
        fo = [const.tile([128, NT], BF16, name=f"fo{i}") for i in range(4)]

        # ---- phase 1: xW = Wih @ x (+bias), bf16
        with tc.tile_pool(name="ps1", bufs=4, space="PSUM") as ps1:
            for d in range(2):
                for g in range(4):
                    for ck in range(NCK):
                        ps = ps1.tile([128, 512], F32, name="p1")
                        nc.tensor.matmul(
                            ps, lhsT=wihT[d][:, 128*g:128*(g+1)],
                            rhs=xT[:, 512*ck:512*(ck+1)],
                            start=True, stop=True)
                        c0 = (2*g + d) * NT + 512*ck
                        nc.vector.tensor_scalar(
                            xw[:, c0:c0+512],
                            ps, bias[d][:, g:g+1], None, OP.add)

        def _dummy_out(*src_aps):
            with tc.tile_pool(name="dumm", bufs=1) as dm:
                for i, ap in enumerate(src_aps):
                    tdum = dm.tile([128, 1], F32, name=f"tdum{i}")
                    nc.vector.tensor_copy(tdum[:], ap)
                    dst = outs['d_out'] if i == 0 else outs['ms_out'][:, i-1:i]
                    nc.sync.dma_start(dst, tdum[:])

        if stop_after <= 1:
            _dummy_out(xw[:, 0:1], xw[:, 8*NT-1:8*NT])
            return

        # ---- phase 2: BiLSTM recurrence. Two independent chains (fwd d=0,
        # bwd d=1) emitted on a hand-placed modulo schedule (half-period
        # offset) so their engine bursts interleave instead of colliding.
        # xw col = g*(2*NT) + d*NT + b*T + t; psum col (per dir) = g*4 + b.
        xw_r = xw.rearrange("p (g d b t) -> p g d b t", g=4, d=2, b=BPC)
        h_r = hst.rearrange("p (d b t) -> p d b t", d=2, b=BPC)
        with tc.tile_pool(name="ps2", bufs=3, space="PSUM") as ps2, \
             tc.tile_pool(name="lwork", bufs=6) as lw:
            cst = {d: const.tile([128, BPC], F32, name=f"c{d}")
                   for d in range(2)}
            for d in range(2):
                nc.vector.memset(cst[d][:], 0.0)

            psums, ghats, shs = {}, {}, {}
            _last = {}

            import os as _os
            _noord = _os.environ.get("K_NOORD") == "1"

            def _ord(key, binst):
                # force per-engine issue order to match emission order
                if _noord:
                    return binst
                prev = _last.get(key)
                if prev is not None:
                    add_dep_helper(binst.ins, prev.ins, sync=False,
                                   reason="modulo-order")
                _last[key] = binst
                return binst

            def part_mm(d, i):
                t = i if d == 0 else T - 1 - i
                tp = t - 1 if d == 0 else t + 1
                ps = ps2.tile([128, 16], F32, name=f"psg{d}", tag=f"psg{d}")
                _ord("PE", nc.tensor.matmul(
                    ps, lhsT=ident[:], rhs=xw_r[:, :, d, :, t],
                    start=True, stop=(i == 0)))
                if i > 0:
                    for g in range(4):
                        _ord("PE", nc.tensor.matmul(
                            ps[:, 4*g:4*(g+1)],
                            lhsT=whhT[d][:, 128*g:128*(g+1)],
                            rhs=h_r[:, d, :, tp],
                            start=False, stop=(g == 3)))
                psums[d] = ps

            def part_sig(d, i):
                ghat = lw.tile([128, 16], F32, name=f"ghat{d}",
                               tag=f"ghat{d}")
                _ord("ACT", nc.scalar.activation(ghat[:], psums[d],
                                                 AF.Sigmoid))
                ghats[d] = ghat

            def part_cpath(d, i):
                ghat = ghats[d]
                u = lw.tile([128, 4], F32, name=f"u{d}", tag=f"u{d}")
                _ord("DVE", nc.vector.tensor_tensor(
                    u[:], ghat[:, 4:8], cst[d][:], OP.mult))
                gt = lw.tile([128, 4], F32, name=f"gt{d}", tag=f"gt{d}")
                _ord("DVE", nc.vector.tensor_scalar(
                    gt[:], ghat[:, 12:16], 2.0, -1.0, OP.mult, OP.add))
                t1 = lw.tile([128, 4], F32, name=f"t1{d}", tag=f"t1{d}")
                _ord("DVE", nc.vector.tensor_tensor(
                    t1[:], ghat[:, 0:4], gt[:], OP.mult))
                _ord("DVE", nc.vector.tensor_tensor(
                    cst[d][:], u[:], t1[:], OP.add))

            def part_sigc(d, i):
                sh = lw.tile([128, 4], F32, name=f"sh{d}", tag=f"sh{d}")
                _ord("ACT", nc.scalar.activation(
                    sh[:], cst[d][:], AF.Tanh))
                shs[d] = sh

            def part_hpath(d, i):
                t = i if d == 0 else T - 1 - i
                _ord("DVE", nc.vector.tensor_tensor(
                    h_r[:, 0 if d == 0 else 1, :, t],
                    ghats[d][:, 8:12], shs[d][:], OP.mult))

            # modulo schedule, one period per step index i (phi order):
            # MM_f(0) hpath_b(135) MM_b(350) sig_f(520) cpath_f(745)
            # sig_b(870) cpath_b(1095) sigc_f(1230) hpath_f(1485) sigc_b(1580)
            part_mm(0, 0)
            part_mm(1, 0)
            part_sig(0, 0)
            part_cpath(0, 0)
            part_sig(1, 0)
            part_cpath(1, 0)
            part_sigc(0, 0)
            part_hpath(0, 0)
            part_sigc(1, 0)
            for i in range(1, T):
                part_mm(0, i)
                part_hpath(1, i - 1)
                part_mm(1, i)
                part_sig(0, i)
                part_cpath(0, i)
                part_sig(1, i)
                part_cpath(1, i)
                part_sigc(0, i)
                part_hpath(0, i)
                part_sigc(1, i)
            part_hpath(1, T - 1)

        if stop_after <= 2:
            _dummy_out(hst[:, NT-1:NT], hst[:, NT:NT+1])
            return

        # ---- phase 3: fo = tanh(Wf enc + b) via 2*sigmoid(2x)-1
        with tc.tile_pool(name="ps3", bufs=4, space="PSUM") as ps3, \
             tc.tile_pool(name="pwork", bufs=3) as pw:
            for ck in range(NCK):
                for isl in range(4):
                    ps = ps3.tile([128, 512], F32, name="p3")
                    nc.tensor.matmul(
                        ps, lhsT=wfT[0][:, 128*isl:128*(isl+1)],
                        rhs=hst[:, 512*ck:512*(ck+1)],
                        start=True, stop=False)
                    nc.tensor.matmul(
                        ps, lhsT=wfT[1][:, 128*isl:128*(isl+1)],
                        rhs=hst[:, NT + 512*ck:NT + 512*(ck+1)],
                        start=False, stop=True)
                    sp_ = pw.tile([128, 512], F32, name="sp", tag="sp")
                    nc.scalar.activation(
                        sp_[:], ps, AF.Sigmoid, bias=bias_fw[:, isl:isl+1])
                    nc.vector.tensor_scalar(
                        fo[isl][:, 512*ck:512*(ck+1)],
                        sp_[:], 2.0, -1.0, OP.mult, OP.add)

        if stop_after <= 3:
            _dummy_out(fo[0][:, 0:1], fo[3][:, NT-1:NT])
            return

        # table-set switch fence (sigmoid set -> exp set)
        tc.strict_bb_all_engine_barrier()

        # ---- phase 4: heads, softmax pieces, gather, relayout to DRAM
        with tc.tile_pool(name="dram", bufs=1, space="DRAM") as dp:
            phD = dp.tile([128, T * PJ], F32, name="phD")
            phD_r = phD.rearrange("r (t j) -> r t j", j=PJ)
            with tc.tile_pool(name="ps4", bufs=2, space="PSUM") as ps4, \
                 tc.tile_pool(name="ps5", bufs=2, space="PSUM") as ps5, \
                 tc.tile_pool(name="ps6", bufs=2, space="PSUM") as ps6, \
                 tc.tile_pool(name="hwork", bufs=2) as hw:
                for ck in range(NCK):
                    psl = ps4.tile([128, 512], F32, name="p4")
                    for k in range(4):
                        nc.tensor.matmul(
                            psl, lhsT=whT[:, 128*k:128*(k+1)],
                            rhs=fo[k][:, 512*ck:512*(ck+1)],
                            start=(k == 0), stop=(k == 3))
                    ut = hw.tile([128, 512], BF16, name="ut", tag="ut")
                    nc.scalar.activation(ut[:], psl, AF.Exp, bias=bias_h[:])
                    psz = ps5.tile([128, 8], F32, name="p5", tag="p5")
                    for tl in range(4):
                        nc.tensor.matmul(
                            psz[:, 2*tl:2*(tl+1)],
                            lhsT=ut[:, 128*tl:128*(tl+1)], rhs=sel[:],
                            start=True, stop=True)
                    zi = hw.tile([128, 8], F32, name="zi", tag="zi")
                    nc.vector.reciprocal(zi[:], psz)
                    for tl in range(4):
                        g0 = ck * 512 + tl * 128
                        bl, t0 = g0 // T, g0 % T
                        for h in range(2):
                            c = h * BPC + bl
                            psg = ps6.tile([128, SP], F32, name="p6",
                                           tag="p6")
                            nc.tensor.matmul(
                                psg,
                                lhsT=ut[64*h:64*(h+1), 128*tl:128*(tl+1)],
                                rhs=E2[64*h:64*(h+1), c*SP:(c+1)*SP],
                                start=True, stop=True)
                            stg = hw.tile([128, SP], F32, name="stg",
                                          tag="stg")
                            nc.vector.tensor_scalar(
                                stg[:], psg, zi[:, 2*tl+h:2*tl+h+1],
                                None, OP.mult)
                            # window w in [2, WIN) <-> stage col OWN*k + w
                            dst = phD_r[c*KSL:(c+1)*KSL, t0:t0+128, :]\
                                .rearrange("k t j -> t k j")
                            src_o = stg[:, HW:HW+KSL*OWN].rearrange(
                                "p (k j) -> p k j", j=OWN)
                            nc.sync.dma_start(
                                dst[:, :, HW-2:PJ], src_o)
                            src_h = stg[:, 2:2+KSL*OWN].rearrange(
                                "p (k j) -> p k j", j=OWN)[:, :, 0:HW-2]
                            nc.sync.dma_start(
                                dst[:, :, 0:HW-2], src_h)

            if stop_after <= 4:
                with tc.tile_pool(name="dum4", bufs=1) as dm:
                    td = dm.tile([128, 2], F32, name="td4")
                    nc.sync.dma_start(td[:, 0:1], phD[:, 0:1])
                    nc.sync.dma_start(td[:, 1:2], phD[:, T*PJ-1:T*PJ])
                    nc.sync.dma_start(outs['d_out'], td[:, 0:1])
                return

            # ---- phase 5: CTC alpha DP, packed layout with recomputed halo
            CH = 64                       # DP steps per streamed chunk
            with tc.tile_pool(name="pch", bufs=2) as pcp, \
                 tc.tile_pool(name="psD", bufs=2, space="PSUM") as psD, \
                 tc.tile_pool(name="dwork", bufs=1) as dw:
                alpha = dw.tile([128, WIN], F32, name="alpha")
                wt = dw.tile([128, WIN], F32, name="wt")
                vt = dw.tile([128, PJ], F32, name="vt")
                ratio = const.tile_from(ins['ratio0'], name="ratio_sb")
                msb = dw.tile([128, NR], F32, name="msb")
                minv = dw.tile([128, 1], F32, name="minv")
                mprev = dw.tile([128, 1], F32, name="mprev")
                d_sb = dw.tile([128, 1], F32, name="d_sb")
                nc.vector.memset(alpha[:], 0.0)
                nc.vector.memset(wt[:], 0.0)
                nc.vector.memset(msb[:], 1.0)

                pch = None
                for t in range(T):
                    if t % CH == 0:
                        pch = pcp.tile([128, PJ * CH], F32, name="pch",
                                       tag="pch")
                        nc.sync.dma_start(
                            pch[:], phD[:, t*PJ:(t+CH)*PJ])
                    pt = pch[:, (t % CH)*PJ:(t % CH + 1)*PJ]
                    if t == 0:
                        nc.vector.tensor_tensor(
                            alpha[:, 2:WIN], pt, initM[:], OP.mult)
                    else:
                        nc.vector.tensor_tensor(
                            vt[:], maskM[:], alpha[:, 0:PJ], OP.mult)
                        nc.vector.tensor_tensor(
                            wt[:, 2:WIN], alpha[:, 2:WIN],
                            alpha[:, 1:WIN-1], OP.add)
                        nc.vector.tensor_tensor(
                            wt[:, 2:WIN], wt[:, 2:WIN], vt[:], OP.add)
                        nc.vector.tensor_tensor(
                            alpha[:, 2:WIN], wt[:, 2:WIN], pt, OP.mult)
                    if t % RENORM == 0 and t > 0 and t < T - 1:
                        if True:
                            r = t // RENORM
                            # renorm down by per-partition max (guarded >= 1)
                            nc.vector.tensor_reduce(
                                msb[:, r:r+1], alpha[:, HW:WIN], AX.X, OP.max)
                            nc.vector.tensor_scalar(
                                msb[:, r:r+1], msb[:, r:r+1], 1.0, None,
                                OP.max)
                            nc.vector.reciprocal(minv[:], msb[:, r:r+1])
                            nc.vector.tensor_scalar(
                                alpha[:], alpha[:], minv[:], None, OP.mult)
                            # ratio[p] *= m'[p-1] * minv[p]  (shift via PE)
                            psm = psD.tile([128, 1], F32, name="psm",
                                           tag="psm")
                            nc.tensor.matmul(
                                psm, lhsT=P1[:], rhs=msb[:, r:r+1],
                                start=True, stop=True)
                            nc.vector.tensor_tensor(
                                mprev[:], ratio[:], psm, OP.mult)
                            nc.vector.tensor_tensor(
                                ratio[:], mprev[:], minv[:], OP.mult)
                        # refresh halo from left neighbour (shift via PE)
                        psh = psD.tile([128, HW], F32, name="psh", tag="psh")
                        nc.tensor.matmul(
                            psh, lhsT=P1[:], rhs=alpha[:, OWN:WIN],
                            start=True, stop=True)
                        nc.vector.tensor_scalar(
                            alpha[:, 0:HW], psh, ratio[:], None, OP.mult)

                nc.vector.scalar_tensor_tensor(
                    wt[:], alpha[:], 1.0, F_big[:], OP.mult, OP.mult,
                    accum_out=d_sb[:])
                nc.sync.dma_start(outs['d_out'], d_sb[:])
                nc.sync.dma_start(outs['ms_out'], msb[:])


# ----------------------------------------------------------------- interface

_CACHE = {}


def _cache_dir():
    import hashlib
    with open(__file__, "rb") as f:
        h = hashlib.blake2b(f.read(), digest_size=6).hexdigest()
    return f"/tmp/brnnctc_kernel_cache_{h}"


_CACHE_DIR = _cache_dir()


class _NcShim:
    """Minimal stand-in for a finalized Bacc object, backed by a BIR module
    deserialized from the on-disk cache (so HLO bytes are identical across
    processes and the jax persistent compilation cache can hit)."""

    def __init__(self, m, partition_name):
        import types as _types
        self.m = m
        self.dbg_addr = None
        self.dbg_callbacks = ()
        self.target_bir_lowering = False
        self.has_collectives = False
        self.trn_type = "TRN2"
        self.partition_id_tensor = (
            _types.SimpleNamespace(name=partition_name)
            if partition_name else None)

    def is_finalized(self):
        return True

    def to_json_bytes(self):
        return mybir.module_to_json_bytes(self.m)


def _get_program():
    if 'nc' in _CACHE:
        return _CACHE['nc']
    import os, json, zlib
    os.makedirs(_CACHE_DIR, exist_ok=True)
    bj = os.path.join(_CACHE_DIR, "bir.json.z")
    mj = os.path.join(_CACHE_DIR, "meta.json")
    if not (os.path.exists(bj) and os.path.exists(mj)):
        nc = build_program()
        pname = (nc.partition_id_tensor.name
                 if nc.partition_id_tensor is not None else None)
        blob = zlib.compress(mybir.module_to_json_bytes(nc.m), 1)
        tmp = bj + ".tmp"
        with open(tmp, "wb") as f:
            f.write(blob)
        os.replace(tmp, bj)
        with open(mj + ".tmp", "w") as f:
            json.dump({"partition": pname}, f)
        os.replace(mj + ".tmp", mj)
    with open(mj) as f:
        meta = json.load(f)
    with open(bj, "rb") as f:
        m = mybir.module_from_json_bytes(zlib.decompress(f.read()))
    shim = _NcShim(m, meta["partition"])
    _CACHE['nc'] = shim
    return shim


def _get_runner():
    if 'runner' in _CACHE:
        return _CACHE['runner']
    import os
    import jax
    from jax.sharding import Mesh, PartitionSpec
    from jax.experimental.shard_map import shard_map
    try:
        jax.config.update("jax_compilation_cache_dir",
                          os.path.join(_CACHE_DIR, "jaxcache"))
        jax.config.update("jax_persistent_cache_min_entry_size_bytes", -1)
        jax.config.update("jax_persistent_cache_min_compile_time_secs", 0)
    except Exception:
        pass
    from concourse.bass2jax import (
        _bass_exec_p, partition_id_tensor, install_neuronx_cc_hook)
    install_neuronx_cc_hook()
    nc = _get_program()
    partition_name = (nc.partition_id_tensor.name
                      if nc.partition_id_tensor else None)
    in_names, out_names, out_avals, zero_shapes = [], [], [], []
    for alloc in nc.m.functions[0].allocations:
        if not isinstance(alloc, mybir.MemoryLocationSet):
            continue
        name = alloc.memorylocations[0].name
        if alloc.kind == "ExternalInput":
            if name != partition_name:
                in_names.append(name)
        elif alloc.kind == "ExternalOutput":
            shape = tuple(alloc.tensor_shape)
            dtype = mybir.dt.np(alloc.dtype)
            out_names.append(name)
            out_avals.append(jax.core.ShapedArray(shape, dtype))
            zero_shapes.append((shape, dtype))
    n_params = len(in_names)
    all_names = list(in_names) + list(out_names)
    if partition_name is not None:
        all_names.append(partition_name)

    def _body(*args):
        operands = list(args)
        if partition_name is not None:
            operands.append(partition_id_tensor())
        outs = _bass_exec_p.bind(
            *operands, out_avals=tuple(out_avals),
            in_names=tuple(all_names), out_names=tuple(out_names),
            lowering_input_output_aliases=(), sim_require_finite=True,
            sim_require_nnan=True, nc=nc)
        return tuple(outs)

    devices = jax.devices()[:NCORES]
    mesh = Mesh(np.asarray(devices), ("core",))
    nin = n_params + len(zero_shapes)
    sharded = jax.jit(
        shard_map(_body, mesh=mesh,
                  in_specs=(PartitionSpec("core"),) * nin,
                  out_specs=(PartitionSpec("core"),) * len(out_names),
                  check_rep=False),
        keep_unused=True)
    zeros = [np.zeros((NCORES * s[0], *s[1:]), d) for s, d in zero_shapes]
    runner = (sharded, in_names, out_names, zeros)
    _CACHE['mesh'] = mesh
    _CACHE['runner'] = runner
    return runner


def kernel(**inputs):
    assert np.all(np.asarray(inputs['inputs_length']) == T), \
        "kernel assumes full-length inputs"
    sharded, in_names, out_names, zeros = _get_runner()
    shared = _prep_shared(inputs)
    in_maps, tls_all = [], []
    for core in range(NCORES):
        m, tls = _prep_core(inputs, core, shared)
        in_maps.append(m)
        tls_all.append(tls)

    concat_in = [np.concatenate([in_maps[c][n] for c in range(NCORES)],
                                axis=0) for n in in_names]
    import jax as _jax
    import time as _time
    t0 = _time.time()
    out_arrs = sharded(*concat_in, *zeros)
    _jax.block_until_ready(out_arrs)
    _CACHE['last_exec_wall_ns'] = int((_time.time() - t0) * 1e9)

    class _Res:
        pass
    res = _Res()
    res.results = []
    for c in range(NCORES):
        dd = {}
        for i, name in enumerate(out_names):
            arr = np.asarray(out_arrs[i])
            rows = arr.shape[0] // NCORES
            dd[name] = arr[c*rows:(c+1)*rows]
        res.results.append(dd)

    base_losses, rle_losses = [], []
    for core in range(NCORES):
        d = np.float64(res.results[core]['d_out'][:, 0])
        ms = np.float64(res.results[core]['ms_out'])
        lam = np.sum(np.log(ms), axis=1)          # [128] log Lambda_p
        tls = tls_all[core]
        for h in range(2):
            for bl in range(BPC):
                c = h * BPC + bl
                dk = d[c*KSL:(c+1)*KSL]
                lk = lam[c*KSL:(c+1)*KSL]
                good = dk > 0
                if not np.any(good):
                    ll = -np.inf
                else:
                    terms = np.log(dk[good]) + lk[good]
                    mx = np.max(terms)
                    ll = mx + np.log(np.sum(np.exp(terms - mx)))
                ll -= T * np.log(C_SCALE)
                loss = -ll / tls[c]
                (base_losses if h == 0 else rle_losses).append(loss)
    return np.asarray(
        [np.mean(base_losses), np.mean(rle_losses)], np.float32)


def last_hw_time_ns():
    return _CACHE.get('last_exec_wall_ns')
